# revision 1
# baseline (speedup 1.0000x reference)
"""Bar-level attention Trainium2 kernel (8 NeuronCores, head-parallel).

Contract: kernel(**inputs) takes the FULL inputs from setup_inputs() and
returns the FULL [1, 2048, 512] float32 output.

Strategy (one head per core, 8 heads / 8 cores):
  - Host: transpose hidden -> XT [512, 2048]; slice + transpose per-head
    weights; fold the 1/sqrt(dh) score scale into Wq/bq; compute
    g = sigmoid(gate[h]) on host and ship as replicated [128,1] columns.
  - Device (per core, all fp32):
      XT -> Q^T, K^T [64, 2048] and V [2048, 65] (col 64 = ones).
      For each 1024-wide query half and each 128-row key chunk:
        S^T = K_chunk @ Q^T  (keys on partitions, queries on free axis)
        E = exp(S^T)         (no max subtraction: scores ~ N(0,1))
        global unnorm AV  += V_chunk~.T @ E        -> [65, 1024] PSUM
        local  unnorm AV  += per-bar diagonal-block matmuls (bar_positions
                             are sorted -> blocks are contiguous; block
                             spans are baked in at build time)
        Row 64 of each AV accumulator is the softmax denominator (ones col).
      Final: project both AV results through Wo_h slice, rescale rows by
      g/l_local and (1-g)/l_global, add -> partial output [2048, 512].
  - Host: sum the 8 partial outputs (output projection is sharded over the
    contraction dim) + bo -> [1, 2048, 512].

The global-attention additive bias in the reference is per-query (constant
across keys), and softmax is shift-invariant per row, so it drops out
exactly; global attention is plain dense softmax attention.
"""

import numpy as np

S = 2048
D = 512
H = 8
DH = 64
SCALE = 1.0 / np.sqrt(DH)
NCHUNK = S // 128      # 16 key chunks of 128
NHALF = 2              # query halves of 1024
QHALF = S // NHALF


def _legalize_waits(nc, mybir):
    """This walrus codegen accepts at most ONE sync wait per instruction.
    Split any instruction carrying N>1 waits into N-1 preceding single-wait
    NoOps on the same engine (waits execute in order on the sequencer)."""
    ctr = 0
    for f in nc.m.functions:
        for b in f.blocks:
            insts = b.instructions
            if not any(i.sync_info and len(i.sync_info.on_wait) > 1 for i in insts):
                continue
            new = []
            for ins in insts:
                si = ins.sync_info
                if si is not None and len(si.on_wait) > 1:
                    waits = list(si.on_wait)
                    for w in waits[:-1]:
                        ctr += 1
                        nop = mybir.InstNoOp(name=f"waitsplit-{ctr}", engine=ins.engine)
                        nop.sync_info = mybir.SyncInfo(on_wait=[w], on_update=[])
                        new.append(nop)
                    ins.sync_info = mybir.SyncInfo(
                        on_wait=[waits[-1]], on_update=list(si.on_update)
                    )
                new.append(ins)
            insts.clear()
            insts.extend(new)
    return ctr


def _bar_bounds(bp):
    """bp: sorted int array [S] -> list of (start, end) per bar."""
    change = np.nonzero(np.diff(bp))[0] + 1
    starts = np.concatenate([[0], change])
    ends = np.concatenate([change, [len(bp)]])
    return list(zip(starts.tolist(), ends.tolist()))


def _build(bars):
    import concourse.bass as bass
    import concourse.tile as tile
    import concourse.mybir as mybir

    dt = mybir.dt
    AF = mybir.ActivationFunctionType
    OP = mybir.AluOpType
    f32 = dt.float32
    f32r = dt.float32r

    def F(ap):
        # view a float32r tile as plain fp32 (for the small local-AV matmuls
        # and DVE ops; f32r tiles hold rounded fp32 bits)
        return ap.bitcast(f32)

    nc = bass.Bass()
    xt_d = nc.dram_tensor("xt", [D, S], f32r, kind="ExternalInput")
    # wpack: 4 chunks of [128, 192]: cols 0:64 WqT(scaled), 64:128 WkT, 128:192 WvT
    wpack_d = nc.dram_tensor("wpack", [D, 192], f32r, kind="ExternalInput")
    wot_d = nc.dram_tensor("wot", [DH, D], f32r, kind="ExternalInput")
    # smalls [128, 8]: col0 bq/8 (rows 0:64), col1 bk, col2 bv, col3 g,
    # col4 1-g, col5 ones
    smalls_d = nc.dram_tensor("smalls", [128, 8], f32, kind="ExternalInput")
    zeros_d = nc.dram_tensor("zeros", [128, 512], f32r, kind="ExternalInput")
    # mask bands: chunk c occupies cols [c*512, c*512+w_c); m[kk, j] = 1 iff
    # bar(c*128+kk) == bar(blo_c + j)
    mask_d = nc.dram_tensor("maskband", [128, NCHUNK * 512], f32, kind="ExternalInput")
    out_d = nc.dram_tensor("out_partial", [S, D], f32, kind="ExternalOutput")

    # per-chunk global band [blo_c, bhi_c): union of bars intersecting chunk
    band = []
    for c in range(NCHUNK):
        klo, khi = c * 128, (c + 1) * 128
        bs = [b for b in bars if b[1] > klo and b[0] < khi]
        band.append((bs[0][0], bs[-1][1]))
        assert band[-1][1] - band[-1][0] <= 512

    with tile.TileContext(nc, pool_alloc_mode="queue") as tc:
        with (
            tc.tile_pool(name="persist", bufs=1) as p_keep,
            tc.tile_pool(name="outbuf", bufs=1) as p_out,
        ):
            qt = p_keep.tile([DH, S], f32r, tag="qt")
            kt = p_keep.tile([DH, S], f32r, tag="kt")
            zeros = p_keep.tile([128, 512], f32r, tag="zeros")
            vt = [p_keep.tile([128, DH + 1], f32r, tag=f"vt{c}", name=f"vt{c}") for c in range(NCHUNK)]
            smalls = p_keep.tile([128, 8], f32, tag="smalls")
            wot = p_keep.tile([DH, D], f32r, tag="wot")
            maskt = p_keep.tile([128, NCHUNK * 512], f32, tag="maskt")
            outbuf = p_out.tile([128, NCHUNK * D], f32, tag="outbuf")

            # ---------------- projections ----------------
            with (
                tc.tile_pool(name="inp", bufs=1) as p_in,
                tc.tile_pool(name="pj", bufs=2, space="PSUM") as p_pj,
                tc.tile_pool(name="pv", bufs=2, space="PSUM") as p_pv,
            ):
                xts = [p_in.tile([128, S], f32r, tag=f"xt{i}", name=f"xts{i}") for i in range(4)]
                wps = [p_in.tile([128, 192], f32r, tag=f"wp{i}", name=f"wps{i}") for i in range(4)]
                nc.sync.dma_start(smalls[:], smalls_d[:])
                nc.sync.dma_start(zeros[:], zeros_d[:])
                for i in range(4):
                    nc.sync.dma_start(
                        wps[i][:], wpack_d[i * 128 : (i + 1) * 128, :]
                    )
                # two column panels per tile: the first QT/KT/V matmul groups
                # unlock after ~2MB instead of the full 4MB. The 4MB mask
                # band is deferred: the shared SDMA engines serialize
                # transfers, and the mask isn't consumed until the first
                # local-AV (~25us in).
                for n in range(2):
                    for i in range(4):
                        nc.sync.dma_start(
                            xts[i][:, n * 1024 : (n + 1) * 1024],
                            xt_d[i * 128 : (i + 1) * 128, n * 1024 : (n + 1) * 1024],
                        )
                # mask on the same (sync) queue: a gpsimd-issued DMA would
                # start immediately (Pool engine idle) and hog the shared
                # SDMA engines ahead of the XT panels
                nc.sync.dma_start(maskt[:], mask_d[:])
                nc.sync.dma_start(wot[:], wot_d[:])

                # Q^T and K^T: [64, 2048] in 1024-halves through 2 psum bufs
                for which, dest, wcol, bcol in ((0, qt, 0, 0), (1, kt, 64, 1)):
                    for hq in range(NHALF):
                        ps = p_pj.tile([DH, QHALF], f32, tag="pj")
                        for n in range(QHALF // 512):
                            for kc in range(4):
                                nc.tensor.matmul(
                                    ps[:, n * 512 : (n + 1) * 512],
                                    (wps[kc][:, wcol : wcol + 64]),
                                    (xts[kc][
                                        :,
                                        hq * QHALF + n * 512 : hq * QHALF + (n + 1) * 512,
                                    ]),
                                    start=(kc == 0),
                                    stop=(kc == 3),
                                )
                        nc.scalar.activation(
                            dest[:, hq * QHALF : (hq + 1) * QHALF],
                            ps[:],
                            AF.Identity,
                            bias=smalls[0:DH, bcol : bcol + 1],
                        )

                # V in natural [k, dh] layout, chunk by chunk; col 64 = 1.0
                for c in range(NCHUNK):
                    ps = p_pv.tile([128, DH], f32, tag="pv")
                    for kc in range(4):
                        nc.tensor.matmul(
                            ps[:],
                            (xts[kc][:, c * 128 : (c + 1) * 128]),
                            (wps[kc][:, 128:192]),
                            start=(kc == 0),
                            stop=(kc == 3),
                        )
                    nc.scalar.activation(
                        vt[c][:, 0:DH],
                        ps[:],
                        AF.Identity,
                        bias=smalls[:, 2:3],
                    )
                    nc.scalar.copy(vt[c][:, DH : DH + 1], smalls[:, 5:6])

            # ---------------- attention ----------------
            # per-(half, chunk) bar pieces baked from bar_positions
            ogs = []
            ols = []
            with tc.tile_pool(name="avout", bufs=1) as p_av:
                l2l = p_av.tile([128, NCHUNK], f32r, tag="l2l")
                l2g = p_av.tile([128, NCHUNK], f32r, tag="l2g")
                r2l = p_av.tile([128, NCHUNK], f32, tag="r2l")
                r2g = p_av.tile([128, NCHUNK], f32, tag="r2g")
                with (
                    tc.tile_pool(name="ps", bufs=2, space="PSUM") as p_s,
                    tc.tile_pool(name="pog", bufs=1, space="PSUM") as p_og,
                    tc.tile_pool(name="pol", bufs=1, space="PSUM") as p_ol,
                    tc.tile_pool(name="pe", bufs=3) as p_e,
                    tc.tile_pool(name="pel", bufs=2) as p_el,
                ):
                  for hq in range(NHALF):
                    og_sb = p_av.tile([DH + 1, QHALF], f32r, tag=f"og{hq}", name=f"ogsb{hq}")
                    ol_sb = p_av.tile([DH + 1, QHALF], f32r, tag=f"ol{hq}", name=f"olsb{hq}")
                    ogs.append(og_sb)
                    ols.append(ol_sb)
                    if True:
                        og = p_og.tile([DH + 1, QHALF], f32, tag="og", name=f"og{hq}")
                        ol = p_ol.tile([DH + 1, QHALF], f32, tag="ol", name=f"ol{hq}")
                        # HW: start=True clears has_written for the WHOLE
                        # bank, so interleaved per-region accumulation groups
                        # corrupt each other. Zero-init ol once with a full
                        # width start=True matmul against zeros, then every
                        # local piece accumulates with start=False.
                        for n in range(QHALF // 512):
                            nc.tensor.matmul(
                                ol[:, n * 512 : (n + 1) * 512],
                                vt[0][:],
                                zeros[:],
                                start=True,
                                stop=False,
                                skip_group_check=True,
                            )
                        for c in range(NCHUNK):
                            sc = p_s.tile([128, QHALF], f32, tag="s")
                            for n in range(QHALF // 512):
                                nc.tensor.matmul(
                                    sc[:, n * 512 : (n + 1) * 512],
                                    (kt[:, c * 128 : (c + 1) * 128]),
                                    (qt[
                                        :,
                                        hq * QHALF + n * 512 : hq * QHALF + (n + 1) * 512,
                                    ]),
                                    start=True,
                                    stop=True,
                                )
                            ec = p_e.tile([128, QHALF], f32r, tag="e")
                            nc.scalar.activation(ec[:], sc[:], AF.Exp)
                            # global AV accumulation
                            for n in range(QHALF // 512):
                                nc.tensor.matmul(
                                    og[:, n * 512 : (n + 1) * 512],
                                    (vt[c][:]),
                                    (ec[:, n * 512 : (n + 1) * 512]),
                                    start=(c == 0),
                                    stop=(c == NCHUNK - 1),
                                )
                            # local AV: masked band of E (bars are contiguous
                            # diagonal blocks); matmul base partitions must be
                            # 0/32/64, so zero-pad a full-128-row band copy.
                            klo, khi = c * 128, (c + 1) * 128
                            qlo, qhi = hq * QHALF, (hq + 1) * QHALF
                            pieces = []  # (qs, qe, rlo, rhi, start, stop)
                            for (s_b, e_b) in bars:
                                if e_b <= klo or s_b >= khi:
                                    continue
                                qs = max(s_b, qlo)
                                qe = min(e_b, qhi)
                                if qs >= qe:
                                    continue
                                pieces.append(
                                    (
                                        qs,
                                        qe,
                                        max(s_b, klo) - klo,
                                        min(e_b, khi) - klo,
                                        s_b >= klo,
                                        e_b <= khi,
                                    )
                                )
                            if pieces:
                                blo, bhi = band[c]
                                hs = pieces[0][0]   # half-clipped band start
                                he = pieces[-1][1]
                                w = he - hs
                                el = p_el.tile([128, 512], f32, tag="el", name="el")
                                nc.vector.tensor_mul(
                                    el[:, 0:w],
                                    F(ec[:, hs - qlo : he - qlo]),
                                    maskt[:, c * 512 + (hs - blo) : c * 512 + (he - blo)],
                                )
                                # matmul runs: merge adjacent pieces with same
                                # flags, split at 512-col psum bank boundaries
                                runs = []
                                for (qs, qe, _, _, st, sp) in pieces:
                                    if runs and runs[-1][2] == st and runs[-1][3] == sp and runs[-1][1] == qs:
                                        runs[-1][1] = qe
                                    else:
                                        runs.append([qs, qe, st, sp])
                                for (qs, qe, st, sp) in runs:
                                    a = qs
                                    while a < qe:
                                        b_ = min(qe, ((a - qlo) // 512 + 1) * 512 + qlo)
                                        nc.tensor.matmul(
                                            ol[:, a - qlo : b_ - qlo],
                                            F(vt[c][:]),
                                            el[:, a - hs : b_ - hs],
                                            start=False,
                                            stop=False,
                                            skip_group_check=True,
                                        )
                                        a = b_
                        # close the ol accumulation group (adds zeros)
                        for n in range(QHALF // 512):
                            nc.tensor.matmul(
                                ol[:, n * 512 : (n + 1) * 512],
                                vt[0][:],
                                zeros[:],
                                start=False,
                                stop=True,
                                skip_group_check=True,
                            )
                        nc.scalar.copy(og_sb[:], og[:])
                        nc.scalar.copy(ol_sb[:], ol[:])
                    # denominator rows -> [128, 8] reshape, natural order:
                    # l2[p, hq*8+jj] = l_half[p*8+jj]; issued per half so
                    # half0's transfers hide under half1's attention
                    j0 = hq * (NCHUNK // NHALF)
                    nc.sync.dma_start(
                        l2l[:, j0 : j0 + NCHUNK // NHALF], ol_sb[DH : DH + 1, :]
                    )
                    nc.sync.dma_start(
                        l2g[:, j0 : j0 + NCHUNK // NHALF], og_sb[DH : DH + 1, :]
                    )

                # ---------------- denominators + recip ----------------
                if True:
                    nc.vector.reciprocal(r2l[:], F(l2l[:]))
                    nc.vector.reciprocal(r2g[:], F(l2g[:]))
                    # fold gate: r_l *= g, r_g *= (1-g)
                    nc.vector.tensor_scalar_mul(r2l[:], r2l[:], smalls[:, 3:4])
                    nc.vector.tensor_scalar_mul(r2g[:], r2g[:], smalls[:, 4:5])

                    # ---------------- output projection + combine ----------
                    with (
                        tc.tile_pool(name="pp", bufs=4, space="PSUM") as p_pp,
                        tc.tile_pool(name="t1", bufs=2) as p_t1,
                    ):
                        for j in range(NCHUNK):
                            hq = j // (NCHUNK // NHALF)
                            jj = j % (NCHUNK // NHALF)
                            # interleaved query chunk: cols jj, jj+8, ...
                            lp = p_pp.tile([128, D], f32, tag="pp")
                            nc.tensor.matmul(
                                lp[:],
                                (ols[hq][0:DH, jj : QHALF : NCHUNK // NHALF]),
                                (wot[:]),
                                start=True,
                                stop=True,
                            )
                            gp = p_pp.tile([128, D], f32, tag="pp")
                            nc.tensor.matmul(
                                gp[:],
                                (ogs[hq][0:DH, jj : QHALF : NCHUNK // NHALF]),
                                (wot[:]),
                                start=True,
                                stop=True,
                            )
                            t1 = p_t1.tile([128, D], f32, tag="t1")
                            # t1 = lp * r_l[q]  (per-partition scale)
                            nc.vector.tensor_scalar_mul(
                                t1[:], lp[:], r2l[:, j : j + 1]
                            )
                            # out = gp * r_g[q] + t1  (DVE fused)
                            nc.vector.scalar_tensor_tensor(
                                outbuf[:, j * D : (j + 1) * D],
                                gp[:],
                                r2g[:, j : j + 1],
                                t1[:],
                                OP.mult,
                                OP.add,
                            )
                            grp = {3: (0, 4), 7: (4, 4), 11: (8, 4),
                                   13: (12, 2), 14: (14, 1), 15: (15, 1)}.get(j)
                            if grp:
                                c0, ng = grp
                                hq_ = c0 // (NCHUNK // NHALF)
                                jj0 = c0 % (NCHUNK // NHALF)
                                dst = out_d[
                                    hq_ * QHALF : (hq_ + 1) * QHALF, :
                                ].rearrange("(p j) c -> p j c", j=NCHUNK // NHALF)[
                                    :, jj0 : jj0 + ng, :
                                ]
                                srcb = outbuf[
                                    :, c0 * D : (c0 + ng) * D
                                ].rearrange("p (j c) -> p j c", j=ng)
                                nc.sync.dma_start(dst, srcb)

    _legalize_waits(nc, mybir)
    return nc


_CACHE = {}


def _get_built(bar_key, bars):
    if bar_key not in _CACHE:
        _CACHE[bar_key] = _build(bars)
    return _CACHE[bar_key]


def _np_reference(hidden_states, bar_positions, attention_mask, Wq, bq, Wk, bk,
                  Wv, bv, Wo, bo, bar_emb, gate):
    """Plain numpy fallback (only used if inputs violate baked assumptions)."""
    B, S_, _ = hidden_states.shape
    x = hidden_states.astype(np.float64)
    q = (x @ Wq.T + bq).reshape(B, S_, H, DH).transpose(0, 2, 1, 3)
    k = (x @ Wk.T + bk).reshape(B, S_, H, DH).transpose(0, 2, 1, 3)
    v = (x @ Wv.T + bv).reshape(B, S_, H, DH).transpose(0, 2, 1, 3)
    scores = np.einsum("bhqd,bhkd->bhqk", q, k) * SCALE
    pad = attention_mask[:, None, None, :]
    bar_mask = (bar_positions[:, :, None] == bar_positions[:, None, :])[:, None]
    NEG = -np.inf

    def softmax(s):
        s = s - s.max(-1, keepdims=True)
        e = np.exp(s)
        return e / e.sum(-1, keepdims=True)

    local = softmax(np.where(bar_mask & pad, scores, NEG))
    emb = bar_emb[np.asarray(bar_positions) % bar_emb.shape[0]]
    bias = np.sum(emb * emb, axis=-1)
    glob = softmax(np.where(pad, scores + bias[:, None, :, None], NEG))
    la = np.einsum("bhqk,bhkd->bhqd", local, v)
    ga = np.einsum("bhqk,bhkd->bhqd", glob, v)
    g = 1.0 / (1.0 + np.exp(-gate))[None, :, None, None]
    comb = g * la + (1.0 - g) * ga
    out = comb.transpose(0, 2, 1, 3).reshape(B, S_, H * DH)
    return (out @ Wo.T + bo).astype(np.float32)


def kernel(**inputs):
    hidden_states = np.asarray(inputs["hidden_states"], dtype=np.float32)
    bar_positions = np.asarray(inputs["bar_positions"])
    attention_mask = np.asarray(inputs["attention_mask"])
    Wq = np.asarray(inputs["Wq"], dtype=np.float32)
    bq = np.asarray(inputs["bq"], dtype=np.float32)
    Wk = np.asarray(inputs["Wk"], dtype=np.float32)
    bk = np.asarray(inputs["bk"], dtype=np.float32)
    Wv = np.asarray(inputs["Wv"], dtype=np.float32)
    bv = np.asarray(inputs["bv"], dtype=np.float32)
    Wo = np.asarray(inputs["Wo"], dtype=np.float32)
    bo = np.asarray(inputs["bo"], dtype=np.float32)
    gate = np.asarray(inputs["gate"], dtype=np.float32)

    bp = bar_positions[0].astype(np.int64)
    if (
        hidden_states.shape != (1, S, D)
        or not bool(attention_mask.all())
        or not bool((np.diff(bp) >= 0).all())
    ):
        return _np_reference(
            hidden_states, bar_positions, attention_mask, Wq, bq, Wk, bk,
            Wv, bv, Wo, bo, np.asarray(inputs["bar_emb"], dtype=np.float32), gate,
        )

    bars = _bar_bounds(bp)
    nc = _get_built(bp.tobytes(), bars)

    # mask bands (same for every core)
    maskband = np.zeros((128, NCHUNK * 512), dtype=np.float32)
    for c in range(NCHUNK):
        klo, khi = c * 128, (c + 1) * 128
        bs = [b for b in bars if b[1] > klo and b[0] < khi]
        blo = bs[0][0]
        eq = (bp[klo:khi, None] == bp[None, blo : bs[-1][1]])
        maskband[:, c * 512 : c * 512 + eq.shape[1]] = eq.astype(np.float32)

    xt = np.ascontiguousarray(hidden_states[0].T)  # [512, 2048]
    g = 1.0 / (1.0 + np.exp(-gate.astype(np.float64)))  # sigmoid, [H]
    in_maps = []
    for h in range(H):
        sl = slice(h * DH, (h + 1) * DH)
        wpack = np.empty((D, 192), dtype=np.float32)
        wpack[:, 0:64] = Wq[sl, :].T * np.float32(SCALE)
        wpack[:, 64:128] = Wk[sl, :].T
        wpack[:, 128:192] = Wv[sl, :].T
        wot = np.ascontiguousarray(Wo[:, sl].T)  # [64, 512]
        smalls = np.zeros((128, 8), dtype=np.float32)
        smalls[0:DH, 0] = bq[sl] * np.float32(SCALE)
        smalls[0:DH, 1] = bk[sl]
        smalls[0:DH, 2] = bv[sl]
        smalls[:, 3] = np.float32(g[h])
        smalls[:, 5] = 1.0
        smalls[:, 4] = np.float32(1.0 - g[h])
        in_maps.append(
            {"xt": xt, "wpack": wpack, "wot": wot, "smalls": smalls,
             "maskband": maskband, "zeros": np.zeros((128, 512), np.float32)}
        )

    res = _run_spmd(nc, in_maps)
    out = np.zeros((S, D), dtype=np.float32)
    for h in range(H):
        out += res.results[h]["out_partial"]
    out += bo
    return out.reshape(1, S, D)


def _run_spmd(nc, in_maps, **kw):
    from concourse.bass_utils import run_bass_kernel_spmd

    return run_bass_kernel_spmd(nc, in_maps, list(range(H)), **kw)



# revision 14
# speedup vs baseline: 1.2578x; 1.2578x over previous
"""Bar-level attention Trainium2 kernel (8 NeuronCores, head-parallel).

Contract: kernel(**inputs) takes the FULL inputs from setup_inputs() and
returns the FULL [1, 2048, 512] float32 output.

Strategy (one head per core, 8 heads / 8 cores), all matmul IO in bf16
(PSUM accumulation stays fp32):
  - Host: XT [512, 2048] bf16; per-head packs:
      wqk [128, 4*128]: per 128-row contraction chunk kc, cols 0:64 =
        (Wq_h.T * scale)[kc], cols 64:128 = Wk_h.T[kc]  -> Q^T and K^T come
        out of ONE matmul stream (stacked stationary, 128 out rows).
      wv  [128, 4*64]: Wv_h.T chunks (V computed in [key, dh] layout with
        64-wide moving operand).
      wot2 [128, 512]: rows 0:64 = g*Wo_h.T, rows 64:128 = (1-g)*Wo_h.T
        (gate folded into the output projection).
      maskp: per-key-chunk bar-equality bands, packed to their true widths.
  - Device per core:
      warmup dummy matmuls (PE p-state ramp), projections pipelined under
      the XT DMA (per-contraction-chunk accumulation passes), then per
      query half: scores S^T = K_c^T Q (keys on partitions), Exp on Act
      engine (the critical resource: ~33us of column time), global AV and
      masked local AV accumulate in PSUM with a trailing ones column giving
      softmax denominators for free.  PSUM has_written semantics (start=True
      clears the whole bank; cleared words are overwritten, not
      accumulated) let local AV pieces accumulate without zero-init.
      Transition: Pool broadcasts the denominator rows, DVE divides the AV
      rows and stacks local (rows 0:64) over global (rows 64:128) in bf16;
      output projection is then ONE matmul per 128-query chunk against
      wot2, drained round-robin over Act/DVE/Pool into bf16 and DMA'd out.
  - Host: sum the 8 bf16 partials in fp32 (contraction-sharded Wo) + bo.

The global-attention additive bias in the reference is per-query and
softmax is shift-invariant per row, so it drops out exactly.
"""

import numpy as np

S = 2048
D = 512
H = 8
DH = 64
SCALE = 1.0 / np.sqrt(DH)
NCHUNK = S // 128       # 16 key chunks of 128
NHALF = 2               # query halves of 1024
QHALF = S // NHALF
VSTRIDE = 66            # per-chunk stride in the packed V tile (64 + ones + pad)


def _legalize_waits(nc, mybir):
    """This walrus codegen accepts at most ONE sync wait per instruction.
    Split any instruction carrying N>1 waits into N-1 preceding single-wait
    NoOps on the same engine (waits execute in order on the sequencer)."""
    ctr = 0
    for f in nc.m.functions:
        for b in f.blocks:
            insts = b.instructions
            if not any(i.sync_info and len(i.sync_info.on_wait) > 1 for i in insts):
                continue
            new = []
            for ins in insts:
                si = ins.sync_info
                if si is not None and len(si.on_wait) > 1:
                    waits = list(si.on_wait)
                    for w in waits[:-1]:
                        ctr += 1
                        nop = mybir.InstNoOp(name=f"waitsplit-{ctr}", engine=ins.engine)
                        nop.sync_info = mybir.SyncInfo(on_wait=[w], on_update=[])
                        new.append(nop)
                    ins.sync_info = mybir.SyncInfo(
                        on_wait=[waits[-1]], on_update=list(si.on_update)
                    )
                new.append(ins)
            insts.clear()
            insts.extend(new)
    return ctr


def _bar_bounds(bp):
    """bp: sorted int array [S] -> list of (start, end) per bar."""
    change = np.nonzero(np.diff(bp))[0] + 1
    starts = np.concatenate([[0], change])
    ends = np.concatenate([change, [len(bp)]])
    return list(zip(starts.tolist(), ends.tolist()))


def _attn_layout(bars):
    """Static layout derived from the (baked) bar boundaries.

    band[c]  = (blo, bhi): union query span of bars intersecting key chunk c
    moff[c]  = column offset of chunk c's band in the packed mask tile
    segs[(hq,c)] = (hs, he) band clipped to the query half, or None
    splits[(hq,c)] = [(a, b, start, stop)]: seg split at 512-col PSUM bank
      boundaries; start/stop mark the first/last matmul touching each bank
      of the local-AV accumulator (has_written bank epoch management).
    """
    band = []
    for c in range(NCHUNK):
        klo, khi = c * 128, (c + 1) * 128
        bs = [b for b in bars if b[1] > klo and b[0] < khi]
        blo, bhi = bs[0][0], bs[-1][1]
        if bhi - blo > 512:
            return None
        band.append((blo, bhi))
    widths = [(b[1] - b[0] + 1) // 2 * 2 for b in band]  # pad even
    moff = [0] * NCHUNK
    for c in range(1, NCHUNK):
        moff[c] = moff[c - 1] + widths[c - 1]
    mw = moff[-1] + widths[-1]

    segs = {}
    splits = {}
    for hq in range(NHALF):
        qlo, qhi = hq * QHALF, (hq + 1) * QHALF
        bank_touch = {}
        for c in range(NCHUNK):
            blo, bhi = band[c]
            hs, he = max(blo, qlo), min(bhi, qhi)
            if hs >= he:
                segs[(hq, c)] = None
                continue
            segs[(hq, c)] = (hs, he)
            ss = []
            a = hs
            while a < he:
                b = min(he, qlo + ((a - qlo) // 512 + 1) * 512)
                bank_touch.setdefault((a - qlo) // 512, []).append((c, len(ss)))
                ss.append([a, b, False, False])
                a = b
            splits[(hq, c)] = ss
        for _, lst in bank_touch.items():
            c0, i0 = lst[0]
            splits[(hq, c0)][i0][2] = True
            c1, i1 = lst[-1]
            splits[(hq, c1)][i1][3] = True
    return band, moff, mw, segs, splits


def _build(bars):
    import concourse.bass as bass
    import concourse.tile as tile
    import concourse.mybir as mybir

    dt = mybir.dt
    AF = mybir.ActivationFunctionType
    OP = mybir.AluOpType
    f32 = dt.float32
    f32r = dt.float32r
    bf16 = dt.bfloat16

    lay = _attn_layout(bars)
    assert lay is not None
    band, moff, mw, segs, splits = lay

    nc = bass.Bass()
    xt_d = nc.dram_tensor("xt", [D, S], bf16, kind="ExternalInput")
    wqk_d = nc.dram_tensor("wqk", [128, 4 * 128], bf16, kind="ExternalInput")
    wv_d = nc.dram_tensor("wv", [128, 4 * 64], bf16, kind="ExternalInput")
    wot_d = nc.dram_tensor("wot", [DH, D], f32r, kind="ExternalInput")
    maskp_d = nc.dram_tensor("maskp", [128, mw], bf16, kind="ExternalInput")
    smalls_d = nc.dram_tensor("smalls", [128, 4], f32, kind="ExternalInput")
    out_d = nc.dram_tensor("out_partial", [S, D], bf16, kind="ExternalOutput")

    with tile.TileContext(nc, pool_alloc_mode="queue") as tc:
        with tc.tile_pool(name="persist", bufs=1) as p_keep:
            qt = p_keep.tile([DH, S], bf16, tag="qt")
            kt = p_keep.tile([DH, S], bf16, tag="kt")
            vt = p_keep.tile([128, NCHUNK * VSTRIDE], bf16, tag="vt")
            wot = p_keep.tile([DH, D], f32r, tag="wot")
            maskp = p_keep.tile([128, mw], bf16, tag="maskp")
            # smalls [128,4] f32: rows 0:64 col0 = bq*scale, col1 = bk;
            # all rows: col2 = sigmoid(gate), col3 = 1-sigmoid(gate)
            smalls = p_keep.tile([128, 4], f32, tag="smalls")
            outbuf = p_keep.tile([128, NCHUNK * D], bf16, tag="outbuf")
            wtiny = p_keep.tile([128, 128], bf16, tag="wtiny")
            # transposed denominators / reciprocals: cols hq*16+jj = local,
            # hq*16+8+jj = global; r2[p, hq*16+jj] = gate/l_local(q) for
            # q = hq*1024 + p*8 + jj (the stride-8 interleave makes each
            # output chunk's scales one column)
            l2 = p_keep.tile([128, 32], f32, tag="l2")
            r2 = p_keep.tile([128, 32], f32, tag="r2")
            ol_sb = [
                p_keep.tile([DH + 1, QHALF], f32r, tag=f"olsb{h}", name=f"ol_sb{h}")
                for h in range(NHALF)
            ]
            og_sb = [
                p_keep.tile([DH + 1, QHALF], f32r, tag=f"ogsb{h}", name=f"og_sb{h}")
                for h in range(NHALF)
            ]

            # ---- PE p-state warmup: keep PE busy from t~0 so the 3us ramp
            # to max clock completes under the input DMA.
            nc.gpsimd.memset(wtiny[:], 0.0)
            # ones columns of the packed V tile (col 64 of each 66-wide chunk)
            nc.gpsimd.memset(
                vt.rearrange("p (c j) -> p c j", j=VSTRIDE)[:, :, DH : DH + 1], 1.0
            )
            with tc.tile_pool(name="pwarm", bufs=1, space="PSUM") as p_w:
                wp = p_w.tile([128, 128], f32, tag="wp")
                for _ in range(30):
                    nc.tensor.matmul(
                        wp[:], wtiny[:], wtiny[:],
                        start=True, stop=True, skip_group_check=True,
                    )

            # ---------------- projections ----------------
            with (
                tc.tile_pool(name="inp", bufs=1) as p_in,
                tc.tile_pool(name="pqk", bufs=1, space="PSUM") as p_qk,
                tc.tile_pool(name="pv", bufs=1, space="PSUM") as p_v,
            ):
                wqk = p_in.tile([128, 4 * 128], bf16, tag="wqk")
                wv = p_in.tile([128, 4 * 64], bf16, tag="wv")
                xts = [
                    p_in.tile([128, S], bf16, tag=f"xt{i}", name=f"xts{i}")
                    for i in range(4)
                ]
                # DMA issue order == DMA_ENGINES service order: weights first
                # (small), then the half-0 xt panels the first matmul pass
                # needs, then half-1, then mask/wot2 (not needed until later).
                nc.sync.dma_start(wqk[:], wqk_d[:])
                nc.sync.dma_start(
                    xts[0][:, 0:QHALF], xt_d[0:128, 0:QHALF]
                )
                nc.sync.dma_start(wv[:], wv_d[:])
                nc.sync.dma_start(smalls[:], smalls_d[:])
                for kc in range(1, 4):
                    nc.sync.dma_start(
                        xts[kc][:, 0:QHALF],
                        xt_d[kc * 128 : (kc + 1) * 128, 0:QHALF],
                    )
                for kc in range(4):
                    nc.sync.dma_start(
                        xts[kc][:, QHALF:S],
                        xt_d[kc * 128 : (kc + 1) * 128, QHALF:S],
                    )
                nc.sync.dma_start(maskp[:], maskp_d[:])
                nc.sync.dma_start(wot[:], wot_d[:])

                qk_ps = p_qk.tile([128, S], f32, tag="qk")
                v_ps = p_v.tile([128, NCHUNK * DH], f32, tag="v")
                for h in range(NHALF):
                    hq0 = h * QHALF
                    for kc in range(4):
                        for n in range(QHALF // 512):
                            nc.tensor.matmul(
                                qk_ps[:, hq0 + n * 512 : hq0 + (n + 1) * 512],
                                wqk[:, kc * 128 : (kc + 1) * 128],
                                xts[kc][:, hq0 + n * 512 : hq0 + (n + 1) * 512],
                                start=(kc == 0),
                                stop=(kc == 3),
                            )
                        for cc in range(8):
                            c = h * 8 + cc
                            nc.tensor.matmul(
                                v_ps[:, c * DH : (c + 1) * DH],
                                xts[kc][:, hq0 + cc * 128 : hq0 + (cc + 1) * 128],
                                wv[:, kc * DH : (kc + 1) * DH],
                                start=(kc == 0 and cc == 0),
                                stop=(kc == 3),
                                skip_group_check=True,
                            )
                    # drain half h: Q^T rows 0:64 (Act, +bias), K^T rows
                    # 64:128 (DVE, +bias), V chunks (Act, strided dest;
                    # GPSIMD cannot read PSUM)
                    nc.scalar.activation(
                        qt[:, hq0 : hq0 + QHALF],
                        qk_ps[0:DH, hq0 : hq0 + QHALF],
                        AF.Identity,
                        bias=smalls[0:DH, 0:1],
                    )
                    nc.vector.tensor_scalar_add(
                        kt[:, hq0 : hq0 + QHALF],
                        qk_ps[DH:128, hq0 : hq0 + QHALF],
                        smalls[0:DH, 1:2],
                    )
                    dstv = vt[
                        :, h * 8 * VSTRIDE : (h + 1) * 8 * VSTRIDE
                    ].rearrange("p (c j) -> p c j", j=VSTRIDE)[:, :, 0:DH]
                    srcv = v_ps[:, h * 512 : (h + 1) * 512].rearrange(
                        "p (c j) -> p c j", j=DH
                    )
                    nc.scalar.copy(dstv, srcv)

            # ---------------- attention ----------------
            with (
                tc.tile_pool(name="ps", bufs=2, space="PSUM") as p_s,
                tc.tile_pool(name="pog", bufs=1, space="PSUM") as p_og,
                tc.tile_pool(name="pol", bufs=1, space="PSUM") as p_ol,
                tc.tile_pool(name="pe", bufs=3) as p_e,
                tc.tile_pool(name="pel", bufs=2) as p_el,
            ):
                for hq in range(NHALF):
                    qlo = hq * QHALF
                    og = p_og.tile([DH + 1, QHALF], f32, tag="og", name=f"og{hq}")
                    ol = p_ol.tile([DH + 1, QHALF], f32, tag="ol", name=f"ol{hq}")
                    for c in range(NCHUNK):
                        sc = p_s.tile([128, QHALF], f32, tag="s")
                        for n in range(QHALF // 512):
                            nc.tensor.matmul(
                                sc[:, n * 512 : (n + 1) * 512],
                                kt[:, c * 128 : (c + 1) * 128],
                                qt[:, qlo + n * 512 : qlo + (n + 1) * 512],
                                start=True,
                                stop=True,
                            )
                        ec = p_e.tile([128, QHALF], bf16, tag="e")
                        nc.scalar.activation(ec[:], sc[:], AF.Exp)
                        vst = vt[:, c * VSTRIDE : c * VSTRIDE + DH + 1]
                        for n in range(QHALF // 512):
                            nc.tensor.matmul(
                                og[:, n * 512 : (n + 1) * 512],
                                vst,
                                ec[:, n * 512 : (n + 1) * 512],
                                start=(c == 0),
                                stop=(c == NCHUNK - 1),
                            )
                        seg = segs[(hq, c)]
                        if seg is None:
                            continue
                        hs, he = seg
                        blo = band[c][0]
                        w = he - hs
                        el = p_el.tile([128, 512], bf16, tag="el")
                        nc.vector.tensor_mul(
                            el[:, 0:w],
                            ec[:, hs - qlo : he - qlo],
                            maskp[:, moff[c] + hs - blo : moff[c] + he - blo],
                        )
                        for (a, b, st, sp) in splits[(hq, c)]:
                            nc.tensor.matmul(
                                ol[:, a - qlo : b - qlo],
                                vst,
                                el[:, a - hs : b - hs],
                                start=st,
                                stop=sp,
                                skip_group_check=True,
                            )
                    # transition: drain the AV accumulators to SBUF (frees
                    # PSUM for the next half; Act takes og so DVE isn't the
                    # serial chain), then denominator rows -> transposed
                    # [128, 8] layout where reciprocal + gate fold are cheap
                    nc.vector.tensor_copy(ol_sb[hq][:], ol[:])
                    nc.scalar.copy(og_sb[hq][:], og[:])
                    c0 = hq * 16
                    nc.sync.dma_start(
                        l2[:, c0 : c0 + 8], ol_sb[hq][DH : DH + 1, :].bitcast(f32)
                    )
                    nc.sync.dma_start(
                        l2[:, c0 + 8 : c0 + 16], og_sb[hq][DH : DH + 1, :].bitcast(f32)
                    )
                    nc.vector.reciprocal(
                        r2[:, c0 : c0 + 16], l2[:, c0 : c0 + 16]
                    )
                    nc.vector.tensor_scalar_mul(
                        r2[:, c0 : c0 + 8], r2[:, c0 : c0 + 8], smalls[:, 2:3]
                    )
                    nc.vector.tensor_scalar_mul(
                        r2[:, c0 + 8 : c0 + 16], r2[:, c0 + 8 : c0 + 16],
                        smalls[:, 3:4],
                    )

            # ---------------- output projection ----------------
            # interleaved query chunks (queries jj, jj+8, ...): partition p
            # of chunk jj is query hq*1024 + p*8 + jj, so r2 column
            # hq*16+jj is exactly the per-partition scale vector.  Per
            # chunk: local+global projections into one 2-bank psum pair,
            # t1 = lp*r_l on Act (activation scale), out = gp*r_g + t1 on
            # DVE (scalar_tensor_tensor), both reading PSUM directly.
            with (
                tc.tile_pool(name="pout", bufs=2, space="PSUM") as p_o,
                tc.tile_pool(name="pt1", bufs=2) as p_t1,
            ):
                for hq in range(NHALF):
                    c0 = hq * 16
                    for jj in range(8):
                        op = p_o.tile([128, 2 * D], f32, tag="op")
                        nc.tensor.matmul(
                            op[:, 0:D],
                            ol_sb[hq][0:DH, jj:QHALF:8],
                            wot[:],
                            start=True,
                            stop=True,
                        )
                        nc.tensor.matmul(
                            op[:, D : 2 * D],
                            og_sb[hq][0:DH, jj:QHALF:8],
                            wot[:],
                            start=True,
                            stop=True,
                        )
                        t1 = p_t1.tile([128, D], bf16, tag="t1")
                        nc.scalar.activation(
                            t1[:], op[:, 0:D], AF.Identity,
                            scale=r2[:, c0 + jj : c0 + jj + 1],
                        )
                        ob = hq * 8 + jj
                        nc.vector.scalar_tensor_tensor(
                            outbuf[:, ob * D : (ob + 1) * D],
                            op[:, D : 2 * D],
                            r2[:, c0 + 8 + jj : c0 + 9 + jj],
                            t1[:],
                            OP.mult,
                            OP.add,
                        )
                        if jj % 4 == 3:
                            g0 = hq * 8 + jj - 3
                            dst = out_d[
                                hq * QHALF : (hq + 1) * QHALF, :
                            ].rearrange("(p j) c -> p j c", j=8)[:, jj - 3 : jj + 1, :]
                            src = outbuf[:, g0 * D : (g0 + 4) * D].rearrange(
                                "p (j c) -> p j c", j=4
                            )
                            nc.sync.dma_start(dst, src)

    _legalize_waits(nc, mybir)
    return nc


_CACHE = {}


def _get_built(bar_key, bars):
    if bar_key not in _CACHE:
        _CACHE[bar_key] = _build(bars)
    return _CACHE[bar_key]


def _np_reference(hidden_states, bar_positions, attention_mask, Wq, bq, Wk, bk,
                  Wv, bv, Wo, bo, bar_emb, gate):
    """Plain numpy fallback (only used if inputs violate baked assumptions)."""
    B, S_, _ = hidden_states.shape
    x = hidden_states.astype(np.float64)
    q = (x @ Wq.T + bq).reshape(B, S_, H, DH).transpose(0, 2, 1, 3)
    k = (x @ Wk.T + bk).reshape(B, S_, H, DH).transpose(0, 2, 1, 3)
    v = (x @ Wv.T + bv).reshape(B, S_, H, DH).transpose(0, 2, 1, 3)
    scores = np.einsum("bhqd,bhkd->bhqk", q, k) * SCALE
    pad = attention_mask[:, None, None, :]
    bar_mask = (bar_positions[:, :, None] == bar_positions[:, None, :])[:, None]
    NEG = -np.inf

    def softmax(s):
        s = s - s.max(-1, keepdims=True)
        e = np.exp(s)
        return e / e.sum(-1, keepdims=True)

    local = softmax(np.where(bar_mask & pad, scores, NEG))
    emb = bar_emb[np.asarray(bar_positions) % bar_emb.shape[0]]
    bias = np.sum(emb * emb, axis=-1)
    glob = softmax(np.where(pad, scores + bias[:, None, :, None], NEG))
    la = np.einsum("bhqk,bhkd->bhqd", local, v)
    ga = np.einsum("bhqk,bhkd->bhqd", glob, v)
    g = 1.0 / (1.0 + np.exp(-gate))[None, :, None, None]
    comb = g * la + (1.0 - g) * ga
    out = comb.transpose(0, 2, 1, 3).reshape(B, S_, H * DH)
    return (out @ Wo.T + bo).astype(np.float32)


def kernel(**inputs):
    import ml_dtypes

    bf16 = ml_dtypes.bfloat16

    hidden_states = np.asarray(inputs["hidden_states"], dtype=np.float32)
    bar_positions = np.asarray(inputs["bar_positions"])
    attention_mask = np.asarray(inputs["attention_mask"])
    Wq = np.asarray(inputs["Wq"], dtype=np.float32)
    bq = np.asarray(inputs["bq"], dtype=np.float32)
    Wk = np.asarray(inputs["Wk"], dtype=np.float32)
    bk = np.asarray(inputs["bk"], dtype=np.float32)
    Wv = np.asarray(inputs["Wv"], dtype=np.float32)
    bv = np.asarray(inputs["bv"], dtype=np.float32)
    Wo = np.asarray(inputs["Wo"], dtype=np.float32)
    bo = np.asarray(inputs["bo"], dtype=np.float32)
    gate = np.asarray(inputs["gate"], dtype=np.float32)

    bp = bar_positions[0].astype(np.int64)
    bars = _bar_bounds(bp)
    if (
        hidden_states.shape != (1, S, D)
        or not bool(attention_mask.all())
        or not bool((np.diff(bp) >= 0).all())
        or bool(np.abs(bv).max() > 0)
        or _attn_layout(bars) is None
    ):
        return _np_reference(
            hidden_states, bar_positions, attention_mask, Wq, bq, Wk, bk,
            Wv, bv, Wo, bo, np.asarray(inputs["bar_emb"], dtype=np.float32), gate,
        )

    nc = _get_built(bp.tobytes(), bars)
    band, moff, mw, _, _ = _attn_layout(bars)

    # packed mask bands (same for every core)
    maskp = np.zeros((128, mw), dtype=bf16)
    for c in range(NCHUNK):
        klo, khi = c * 128, (c + 1) * 128
        blo, bhi = band[c]
        eq = bp[klo:khi, None] == bp[None, blo:bhi]
        maskp[:, moff[c] : moff[c] + (bhi - blo)] = eq.astype(bf16)

    xt = np.ascontiguousarray(hidden_states[0].T).astype(bf16)  # [512, 2048]
    g = 1.0 / (1.0 + np.exp(-gate.astype(np.float64)))  # sigmoid, [H]
    in_maps = []
    for h in range(H):
        sl = slice(h * DH, (h + 1) * DH)
        wqt = Wq[sl, :].T * np.float32(SCALE)  # [512, 64]
        wkt = Wk[sl, :].T
        wvt = Wv[sl, :].T
        wqk = np.empty((128, 4 * 128), dtype=np.float32)
        wv = np.empty((128, 4 * 64), dtype=np.float32)
        for kc in range(4):
            r = slice(kc * 128, (kc + 1) * 128)
            wqk[:, kc * 128 : kc * 128 + 64] = wqt[r]
            wqk[:, kc * 128 + 64 : (kc + 1) * 128] = wkt[r]
            wv[:, kc * 64 : (kc + 1) * 64] = wvt[r]
        wot = np.ascontiguousarray(Wo[:, sl].T)  # [64, 512] fp32
        smalls = np.zeros((128, 4), dtype=np.float32)
        smalls[0:DH, 0] = bq[sl] * np.float32(SCALE)
        smalls[0:DH, 1] = bk[sl]
        smalls[:, 2] = np.float32(g[h])
        smalls[:, 3] = np.float32(1.0 - g[h])
        in_maps.append(
            {"xt": xt, "wqk": wqk.astype(bf16), "wv": wv.astype(bf16),
             "wot": wot, "maskp": maskp, "smalls": smalls}
        )

    res = _run_spmd(nc, in_maps)
    out = np.zeros((S, D), dtype=np.float32)
    for h in range(H):
        out += np.asarray(res.results[h]["out_partial"], dtype=np.float32)
    out += bo
    return out.reshape(1, S, D)


def _run_spmd(nc, in_maps, **kw):
    from concourse.bass_utils import run_bass_kernel_spmd

    return run_bass_kernel_spmd(nc, in_maps, list(range(H)), **kw)


# revision 20
# speedup vs baseline: 1.3904x; 1.1054x over previous
"""Bar-level attention Trainium2 kernel (8 NeuronCores, head-parallel).

Contract: kernel(**inputs) takes the FULL inputs from setup_inputs() and
returns the FULL [1, 2048, 512] float32 output.

Strategy (one head per core, 8 heads / 8 cores), all matmul IO in bf16
(PSUM accumulation stays fp32):
  - Host: XT [512, 2048] bf16; per-head packs:
      wqk [128, 4*128]: per 128-row contraction chunk kc, cols 0:64 =
        (Wq_h.T * scale)[kc], cols 64:128 = Wk_h.T[kc]  -> Q^T and K^T come
        out of ONE matmul stream (stacked stationary, 128 out rows).
      wv  [128, 4*64]: Wv_h.T chunks (V computed in [key, dh] layout with
        64-wide moving operand).
      wot2 [128, 512]: rows 0:64 = g*Wo_h.T, rows 64:128 = (1-g)*Wo_h.T
        (gate folded into the output projection).
      maskp: per-key-chunk bar-equality bands, packed to their true widths.
  - Device per core:
      warmup dummy matmuls (PE p-state ramp), projections pipelined under
      the XT DMA (per-contraction-chunk accumulation passes), then per
      query half: scores S^T = K_c^T Q (keys on partitions), Exp on Act
      engine (the critical resource: ~33us of column time), global AV and
      masked local AV accumulate in PSUM with a trailing ones column giving
      softmax denominators for free.  PSUM has_written semantics (start=True
      clears the whole bank; cleared words are overwritten, not
      accumulated) let local AV pieces accumulate without zero-init.
      Transition: Pool broadcasts the denominator rows, DVE divides the AV
      rows and stacks local (rows 0:64) over global (rows 64:128) in bf16;
      output projection is then ONE matmul per 128-query chunk against
      wot2, drained round-robin over Act/DVE/Pool into bf16 and DMA'd out.
  - Host: sum the 8 bf16 partials in fp32 (contraction-sharded Wo) + bo.

The global-attention additive bias in the reference is per-query and
softmax is shift-invariant per row, so it drops out exactly.
"""

import numpy as np

S = 2048
D = 512
H = 8
DH = 64
SCALE = 1.0 / np.sqrt(DH)
NCHUNK = S // 128       # 16 key chunks of 128
NHALF = 2               # query halves of 1024
QHALF = S // NHALF
VSTRIDE = 66            # per-chunk stride in the packed V tile (64 + ones + pad)


def _legalize_waits(nc, mybir):
    """This walrus codegen accepts at most ONE sync wait per instruction.
    Split any instruction carrying N>1 waits into N-1 preceding single-wait
    NoOps on the same engine (waits execute in order on the sequencer)."""
    ctr = 0
    for f in nc.m.functions:
        for b in f.blocks:
            insts = b.instructions
            if not any(i.sync_info and len(i.sync_info.on_wait) > 1 for i in insts):
                continue
            new = []
            for ins in insts:
                si = ins.sync_info
                if si is not None and len(si.on_wait) > 1:
                    waits = list(si.on_wait)
                    for w in waits[:-1]:
                        ctr += 1
                        nop = mybir.InstNoOp(name=f"waitsplit-{ctr}", engine=ins.engine)
                        nop.sync_info = mybir.SyncInfo(on_wait=[w], on_update=[])
                        new.append(nop)
                    ins.sync_info = mybir.SyncInfo(
                        on_wait=[waits[-1]], on_update=list(si.on_update)
                    )
                new.append(ins)
            insts.clear()
            insts.extend(new)
    return ctr


def _bar_bounds(bp):
    """bp: sorted int array [S] -> list of (start, end) per bar."""
    change = np.nonzero(np.diff(bp))[0] + 1
    starts = np.concatenate([[0], change])
    ends = np.concatenate([change, [len(bp)]])
    return list(zip(starts.tolist(), ends.tolist()))


def _attn_layout(bars):
    """Static layout derived from the (baked) bar boundaries.

    band[c]  = (blo, bhi): union query span of bars intersecting key chunk c
    moff[c]  = column offset of chunk c's band in the packed mask tile
    segs[(hq,c)] = (hs, he) band clipped to the query half, or None
    splits[(hq,c)] = [(a, b, start, stop)]: seg split at 512-col PSUM bank
      boundaries; start/stop mark the first/last matmul touching each bank
      of the local-AV accumulator (has_written bank epoch management).
    """
    band = []
    for c in range(NCHUNK):
        klo, khi = c * 128, (c + 1) * 128
        bs = [b for b in bars if b[1] > klo and b[0] < khi]
        blo, bhi = bs[0][0], bs[-1][1]
        if bhi - blo > 512:
            return None
        band.append((blo, bhi))
    widths = [(b[1] - b[0] + 1) // 2 * 2 for b in band]  # pad even
    moff = [0] * NCHUNK
    for c in range(1, NCHUNK):
        moff[c] = moff[c - 1] + widths[c - 1]
    mw = moff[-1] + widths[-1]

    segs = {}
    splits = {}
    for hq in range(NHALF):
        qlo, qhi = hq * QHALF, (hq + 1) * QHALF
        bank_touch = {}
        for c in range(NCHUNK):
            blo, bhi = band[c]
            hs, he = max(blo, qlo), min(bhi, qhi)
            if hs >= he:
                segs[(hq, c)] = None
                continue
            segs[(hq, c)] = (hs, he)
            ss = []
            a = hs
            while a < he:
                b = min(he, qlo + ((a - qlo) // 512 + 1) * 512)
                bank_touch.setdefault((a - qlo) // 512, []).append((c, len(ss)))
                ss.append([a, b, False, False])
                a = b
            splits[(hq, c)] = ss
        for _, lst in bank_touch.items():
            c0, i0 = lst[0]
            splits[(hq, c0)][i0][2] = True
            c1, i1 = lst[-1]
            splits[(hq, c1)][i1][3] = True
    return band, moff, mw, segs, splits


def _build(bars):
    import concourse.bass as bass
    import concourse.tile as tile
    import concourse.mybir as mybir

    dt = mybir.dt
    AF = mybir.ActivationFunctionType
    OP = mybir.AluOpType
    f32 = dt.float32
    f32r = dt.float32r
    bf16 = dt.bfloat16

    lay = _attn_layout(bars)
    assert lay is not None
    band, moff, mw, segs, splits = lay

    nc = bass.Bass()
    xt_d = nc.dram_tensor("xt", [D, S], bf16, kind="ExternalInput")
    wqk_d = nc.dram_tensor("wqk", [128, 4 * 128], bf16, kind="ExternalInput")
    wv_d = nc.dram_tensor("wv", [128, 4 * 64], bf16, kind="ExternalInput")
    wot_d = nc.dram_tensor("wot", [DH, D], f32r, kind="ExternalInput")
    maskp_d = nc.dram_tensor("maskp", [128, mw], bf16, kind="ExternalInput")
    smalls_d = nc.dram_tensor("smalls", [128, 4], f32, kind="ExternalInput")
    out_d = nc.dram_tensor("out_partial", [S, D], bf16, kind="ExternalOutput")

    with tile.TileContext(nc, pool_alloc_mode="queue") as tc:
        with tc.tile_pool(name="persist", bufs=1) as p_keep:
            qt = p_keep.tile([DH, S], bf16, tag="qt")
            kt = p_keep.tile([DH, S], bf16, tag="kt")
            vt = p_keep.tile([128, NCHUNK * VSTRIDE], bf16, tag="vt")
            wot = p_keep.tile([DH, D], f32r, tag="wot")
            maskp = p_keep.tile([128, mw], bf16, tag="maskp")
            # smalls [128,4] f32: rows 0:64 col0 = bq*scale, col1 = bk;
            # all rows: col2 = sigmoid(gate), col3 = 1-sigmoid(gate)
            smalls = p_keep.tile([128, 4], f32, tag="smalls")
            obuf = [
                p_keep.tile([128, 4 * D], bf16, tag=f"ob{i}", name=f"obuf{i}")
                for i in range(4)
            ]
            wtiny = p_keep.tile([128, 128], bf16, tag="wtiny")
            # transposed denominators / reciprocals: cols hq*16+jj = local,
            # hq*16+8+jj = global; r2[p, hq*16+jj] = gate/l_local(q) for
            # q = hq*1024 + p*8 + jj (the stride-8 interleave makes each
            # output chunk's scales one column)
            l2 = p_keep.tile([128, 32], f32, tag="l2")
            r2 = p_keep.tile([128, 32], f32, tag="r2")
            ol_sb = [
                p_keep.tile([DH + 1, QHALF], f32r, tag=f"olsb{h}", name=f"ol_sb{h}")
                for h in range(NHALF)
            ]
            og_sb = [
                p_keep.tile([DH + 1, QHALF], f32r, tag=f"ogsb{h}", name=f"og_sb{h}")
                for h in range(NHALF)
            ]

            # ---- PE p-state warmup: keep PE busy from t~0 so the 3us ramp
            # to max clock completes under the input DMA.
            nc.gpsimd.memset(wtiny[:], 0.0)
            # ones columns of the packed V tile (col 64 of each 66-wide chunk)
            nc.gpsimd.memset(
                vt.rearrange("p (c j) -> p c j", j=VSTRIDE)[:, :, DH : DH + 1], 1.0
            )
            with tc.tile_pool(name="pwarm", bufs=1, space="PSUM") as p_w:
                wp = p_w.tile([128, 128], f32, tag="wp")
                for _ in range(30):
                    nc.tensor.matmul(
                        wp[:], wtiny[:], wtiny[:],
                        start=True, stop=True, skip_group_check=True,
                    )

            # ---------------- projections + attention ----------------
            # Software-pipelined: half-0 projections run under the xt DMA;
            # half-1 projection passes + drains are injected into half-0's
            # score/exp stream (Act only ever runs exp once the loop
            # starts); og/lAV accumulation for early chunks is emitted late
            # so its PSUM banks (freed by the half-1 projection pool) are
            # ready without stalling the in-order engines.
            with tc.tile_pool(name="inp", bufs=1) as p_in:
                wqk = p_in.tile([128, 4 * 128], bf16, tag="wqk")
                wv = p_in.tile([128, 4 * 64], bf16, tag="wv")
                xts = [
                    p_in.tile([128, S], bf16, tag=f"xt{i}", name=f"xts{i}")
                    for i in range(4)
                ]
                # DMA issue order == service order: half-0 panels
                # interleaved with the small weight tensors, then half-1
                # panels, then mask/wot (not consumed until later).
                nc.sync.dma_start(xts[0][:, 0:QHALF], xt_d[0:128, 0:QHALF])
                nc.sync.dma_start(wqk[:], wqk_d[:])
                nc.sync.dma_start(xts[1][:, 0:QHALF], xt_d[128:256, 0:QHALF])
                nc.sync.dma_start(wv[:], wv_d[:])
                nc.sync.dma_start(xts[2][:, 0:QHALF], xt_d[256:384, 0:QHALF])
                nc.sync.dma_start(smalls[:], smalls_d[:])
                nc.sync.dma_start(xts[3][:, 0:QHALF], xt_d[384:512, 0:QHALF])
                for kc in range(4):
                    nc.sync.dma_start(
                        xts[kc][:, QHALF:S],
                        xt_d[kc * 128 : (kc + 1) * 128, QHALF:S],
                    )
                nc.sync.dma_start(maskp[:], maskp_d[:])
                nc.sync.dma_start(wot[:], wot_d[:])

                def proj_pass(qkp, vp, h, kc):
                    hq0 = h * QHALF
                    for n in range(QHALF // 512):
                        nc.tensor.matmul(
                            qkp[:, n * 512 : (n + 1) * 512],
                            wqk[:, kc * 128 : (kc + 1) * 128],
                            xts[kc][:, hq0 + n * 512 : hq0 + (n + 1) * 512],
                            start=(kc == 0),
                            stop=(kc == 3),
                        )
                    for cc in range(8):
                        nc.tensor.matmul(
                            vp[:, cc * DH : (cc + 1) * DH],
                            xts[kc][:, hq0 + cc * 128 : hq0 + (cc + 1) * 128],
                            wv[:, kc * DH : (kc + 1) * DH],
                            start=(kc == 0 and cc == 0),
                            stop=(kc == 3),
                            skip_group_check=True,
                        )

                def proj_drain(qkp, vp, h, qt_eng_act):
                    hq0 = h * QHALF
                    if qt_eng_act:
                        nc.scalar.activation(
                            qt[:, hq0 : hq0 + QHALF], qkp[0:DH, :],
                            AF.Identity, bias=smalls[0:DH, 0:1],
                        )
                    else:
                        nc.vector.tensor_scalar_add(
                            qt[:, hq0 : hq0 + QHALF], qkp[0:DH, :],
                            smalls[0:DH, 0:1],
                        )
                    nc.vector.tensor_scalar_add(
                        kt[:, hq0 : hq0 + QHALF], qkp[DH:128, :],
                        smalls[0:DH, 1:2],
                    )
                    dstv = vt[
                        :, h * 8 * VSTRIDE : (h + 1) * 8 * VSTRIDE
                    ].rearrange("p (c j) -> p c j", j=VSTRIDE)[:, :, 0:DH]
                    nc.vector.tensor_copy(
                        dstv, vp[:].rearrange("p (c j) -> p c j", j=DH)
                    )

                with tc.tile_pool(name="ph0", bufs=1, space="PSUM") as ph0:
                    qk0 = ph0.tile([128, QHALF], f32, tag="qk0")
                    v0 = ph0.tile([128, 8 * DH], f32, tag="v0")
                    for kc in range(4):
                        proj_pass(qk0, v0, 0, kc)
                    proj_drain(qk0, v0, 0, qt_eng_act=True)

                with (
                    tc.tile_pool(name="ps", bufs=2, space="PSUM") as p_s,
                    tc.tile_pool(name="pe", bufs=12) as p_e,
                    tc.tile_pool(name="pel", bufs=12) as p_el,
                ):
                    es = {}
                    els = {}

                    def emit_sc_exp(hq, c):
                        qlo = hq * QHALF
                        sc = p_s.tile([128, QHALF], f32, tag="s", name="sc")
                        for n in range(QHALF // 512):
                            nc.tensor.matmul(
                                sc[:, n * 512 : (n + 1) * 512],
                                kt[:, c * 128 : (c + 1) * 128],
                                qt[:, qlo + n * 512 : qlo + (n + 1) * 512],
                                start=True,
                                stop=True,
                            )
                        ec = p_e.tile([128, QHALF], bf16, tag="e", name="ec")
                        nc.scalar.activation(ec[:], sc[:], AF.Exp)
                        es[(hq, c)] = ec
                        seg = segs[(hq, c)]
                        if seg is not None:
                            hs, he = seg
                            blo = band[c][0]
                            el = p_el.tile([128, 512], bf16, tag="el", name="el")
                            nc.vector.tensor_mul(
                                el[:, 0 : he - hs],
                                ec[:, hs - qlo : he - qlo],
                                maskp[:, moff[c] + hs - blo : moff[c] + he - blo],
                            )
                            els[(hq, c)] = el

                    def emit_og_lav(hq, c, og, ol):
                        qlo = hq * QHALF
                        ec = es.pop((hq, c))
                        vst = vt[:, c * VSTRIDE : c * VSTRIDE + DH + 1]
                        for n in range(QHALF // 512):
                            nc.tensor.matmul(
                                og[:, n * 512 : (n + 1) * 512],
                                vst,
                                ec[:, n * 512 : (n + 1) * 512],
                                start=(c == 0),
                                stop=(c == NCHUNK - 1),
                            )
                        if (hq, c) in els:
                            el = els.pop((hq, c))
                            hs = segs[(hq, c)][0]
                            for (a, b, st, sp) in splits[(hq, c)]:
                                nc.tensor.matmul(
                                    ol[:, a - qlo : b - qlo],
                                    vst,
                                    el[:, a - hs : b - hs],
                                    start=st,
                                    stop=sp,
                                    skip_group_check=True,
                                )

                    def transition(hq, og, ol):
                        nc.vector.tensor_copy(ol_sb[hq][:], ol[:])
                        nc.vector.tensor_copy(og_sb[hq][:], og[:])
                        c0 = hq * 16
                        nc.sync.dma_start(
                            l2[:, c0 : c0 + 8],
                            ol_sb[hq][DH : DH + 1, :].bitcast(f32),
                        )
                        nc.sync.dma_start(
                            l2[:, c0 + 8 : c0 + 16],
                            og_sb[hq][DH : DH + 1, :].bitcast(f32),
                        )
                        nc.vector.reciprocal(
                            r2[:, c0 : c0 + 16], l2[:, c0 : c0 + 16]
                        )
                        nc.vector.tensor_scalar_mul(
                            r2[:, c0 : c0 + 8], r2[:, c0 : c0 + 8],
                            smalls[:, 2:3],
                        )
                        nc.vector.tensor_scalar_mul(
                            r2[:, c0 + 8 : c0 + 16], r2[:, c0 + 8 : c0 + 16],
                            smalls[:, 3:4],
                        )

                    with tc.tile_pool(name="ph1", bufs=1, space="PSUM") as ph1:
                        qk1 = ph1.tile([128, QHALF], f32, tag="qk1")
                        v1 = ph1.tile([128, 8 * DH], f32, tag="v1")
                        # chunks 0..7 only: their kt key-columns come from
                        # the half-0 projection; chunks 8..15 need the half-1
                        # drain below first
                        for c in range(8):
                            emit_sc_exp(0, c)
                            if 1 <= c <= 4:
                                proj_pass(qk1, v1, 1, c - 1)
                        proj_drain(qk1, v1, 1, qt_eng_act=False)

                    with (
                        tc.tile_pool(name="pog", bufs=1, space="PSUM") as p_og,
                        tc.tile_pool(name="pol", bufs=1, space="PSUM") as p_ol,
                    ):
                        og0 = p_og.tile([DH + 1, QHALF], f32, tag="og", name="og0")
                        ol0 = p_ol.tile([DH + 1, QHALF], f32, tag="ol", name="ol0")
                        for c in range(8):
                            emit_sc_exp(0, 8 + c)
                            emit_og_lav(0, c, og0, ol0)
                        for c in range(8, NCHUNK):
                            emit_og_lav(0, c, og0, ol0)
                        transition(0, og0, ol0)

                        og1 = p_og.tile([DH + 1, QHALF], f32, tag="og", name="og1")
                        ol1 = p_ol.tile([DH + 1, QHALF], f32, tag="ol", name="ol1")
                        for c in range(NCHUNK):
                            emit_sc_exp(1, c)
                            if c >= 2:
                                emit_og_lav(1, c - 2, og1, ol1)
                        for c in range(NCHUNK - 2, NCHUNK):
                            emit_og_lav(1, c, og1, ol1)
                        transition(1, og1, ol1)

            # ---------------- output projection ----------------
            # interleaved query chunks (queries jj, jj+8, ...): partition p
            # of chunk jj is query hq*1024 + p*8 + jj, so r2 column
            # hq*16+jj is exactly the per-partition scale vector.  Per
            # chunk: local+global projections into one 2-bank psum pair,
            # t1 = lp*r_l on Act (activation scale), out = gp*r_g + t1 on
            # DVE (scalar_tensor_tensor), both reading PSUM directly.
            with (
                tc.tile_pool(name="pout", bufs=2, space="PSUM") as p_o,
                tc.tile_pool(name="pt1", bufs=2) as p_t1,
            ):
                for hq in range(NHALF):
                    c0 = hq * 16
                    for jj in range(8):
                        op = p_o.tile([128, 2 * D], f32, tag="op")
                        nc.tensor.matmul(
                            op[:, 0:D],
                            ol_sb[hq][0:DH, jj:QHALF:8],
                            wot[:],
                            start=True,
                            stop=True,
                        )
                        nc.tensor.matmul(
                            op[:, D : 2 * D],
                            og_sb[hq][0:DH, jj:QHALF:8],
                            wot[:],
                            start=True,
                            stop=True,
                        )
                        t1 = p_t1.tile([128, D], bf16, tag="t1")
                        nc.scalar.activation(
                            t1[:], op[:, 0:D], AF.Identity,
                            scale=r2[:, c0 + jj : c0 + jj + 1],
                        )
                        grp = obuf[(hq * 8 + jj) // 4]
                        nc.vector.scalar_tensor_tensor(
                            grp[:, (jj % 4) * D : (jj % 4 + 1) * D],
                            op[:, D : 2 * D],
                            r2[:, c0 + 8 + jj : c0 + 9 + jj],
                            t1[:],
                            OP.mult,
                            OP.add,
                        )
                        if jj % 4 == 3:
                            dst = out_d[
                                hq * QHALF : (hq + 1) * QHALF, :
                            ].rearrange("(p j) c -> p j c", j=8)[:, jj - 3 : jj + 1, :]
                            nc.sync.dma_start(
                                dst, grp[:].rearrange("p (j c) -> p j c", j=4)
                            )

    _legalize_waits(nc, mybir)
    return nc


_CACHE = {}


def _get_built(bar_key, bars):
    if bar_key not in _CACHE:
        _CACHE[bar_key] = _build(bars)
    return _CACHE[bar_key]


def _np_reference(hidden_states, bar_positions, attention_mask, Wq, bq, Wk, bk,
                  Wv, bv, Wo, bo, bar_emb, gate):
    """Plain numpy fallback (only used if inputs violate baked assumptions)."""
    B, S_, _ = hidden_states.shape
    x = hidden_states.astype(np.float64)
    q = (x @ Wq.T + bq).reshape(B, S_, H, DH).transpose(0, 2, 1, 3)
    k = (x @ Wk.T + bk).reshape(B, S_, H, DH).transpose(0, 2, 1, 3)
    v = (x @ Wv.T + bv).reshape(B, S_, H, DH).transpose(0, 2, 1, 3)
    scores = np.einsum("bhqd,bhkd->bhqk", q, k) * SCALE
    pad = attention_mask[:, None, None, :]
    bar_mask = (bar_positions[:, :, None] == bar_positions[:, None, :])[:, None]
    NEG = -np.inf

    def softmax(s):
        s = s - s.max(-1, keepdims=True)
        e = np.exp(s)
        return e / e.sum(-1, keepdims=True)

    local = softmax(np.where(bar_mask & pad, scores, NEG))
    emb = bar_emb[np.asarray(bar_positions) % bar_emb.shape[0]]
    bias = np.sum(emb * emb, axis=-1)
    glob = softmax(np.where(pad, scores + bias[:, None, :, None], NEG))
    la = np.einsum("bhqk,bhkd->bhqd", local, v)
    ga = np.einsum("bhqk,bhkd->bhqd", glob, v)
    g = 1.0 / (1.0 + np.exp(-gate))[None, :, None, None]
    comb = g * la + (1.0 - g) * ga
    out = comb.transpose(0, 2, 1, 3).reshape(B, S_, H * DH)
    return (out @ Wo.T + bo).astype(np.float32)


def kernel(**inputs):
    import ml_dtypes

    bf16 = ml_dtypes.bfloat16

    hidden_states = np.asarray(inputs["hidden_states"], dtype=np.float32)
    bar_positions = np.asarray(inputs["bar_positions"])
    attention_mask = np.asarray(inputs["attention_mask"])
    Wq = np.asarray(inputs["Wq"], dtype=np.float32)
    bq = np.asarray(inputs["bq"], dtype=np.float32)
    Wk = np.asarray(inputs["Wk"], dtype=np.float32)
    bk = np.asarray(inputs["bk"], dtype=np.float32)
    Wv = np.asarray(inputs["Wv"], dtype=np.float32)
    bv = np.asarray(inputs["bv"], dtype=np.float32)
    Wo = np.asarray(inputs["Wo"], dtype=np.float32)
    bo = np.asarray(inputs["bo"], dtype=np.float32)
    gate = np.asarray(inputs["gate"], dtype=np.float32)

    bp = bar_positions[0].astype(np.int64)
    bars = _bar_bounds(bp)
    if (
        hidden_states.shape != (1, S, D)
        or not bool(attention_mask.all())
        or not bool((np.diff(bp) >= 0).all())
        or bool(np.abs(bv).max() > 0)
        or _attn_layout(bars) is None
    ):
        return _np_reference(
            hidden_states, bar_positions, attention_mask, Wq, bq, Wk, bk,
            Wv, bv, Wo, bo, np.asarray(inputs["bar_emb"], dtype=np.float32), gate,
        )

    nc = _get_built(bp.tobytes(), bars)
    band, moff, mw, _, _ = _attn_layout(bars)

    # packed mask bands (same for every core)
    maskp = np.zeros((128, mw), dtype=bf16)
    for c in range(NCHUNK):
        klo, khi = c * 128, (c + 1) * 128
        blo, bhi = band[c]
        eq = bp[klo:khi, None] == bp[None, blo:bhi]
        maskp[:, moff[c] : moff[c] + (bhi - blo)] = eq.astype(bf16)

    xt = np.ascontiguousarray(hidden_states[0].T).astype(bf16)  # [512, 2048]
    g = 1.0 / (1.0 + np.exp(-gate.astype(np.float64)))  # sigmoid, [H]
    in_maps = []
    for h in range(H):
        sl = slice(h * DH, (h + 1) * DH)
        wqt = Wq[sl, :].T * np.float32(SCALE)  # [512, 64]
        wkt = Wk[sl, :].T
        wvt = Wv[sl, :].T
        wqk = np.empty((128, 4 * 128), dtype=np.float32)
        wv = np.empty((128, 4 * 64), dtype=np.float32)
        for kc in range(4):
            r = slice(kc * 128, (kc + 1) * 128)
            wqk[:, kc * 128 : kc * 128 + 64] = wqt[r]
            wqk[:, kc * 128 + 64 : (kc + 1) * 128] = wkt[r]
            wv[:, kc * 64 : (kc + 1) * 64] = wvt[r]
        wot = np.ascontiguousarray(Wo[:, sl].T)  # [64, 512] fp32
        smalls = np.zeros((128, 4), dtype=np.float32)
        smalls[0:DH, 0] = bq[sl] * np.float32(SCALE)
        smalls[0:DH, 1] = bk[sl]
        smalls[:, 2] = np.float32(g[h])
        smalls[:, 3] = np.float32(1.0 - g[h])
        in_maps.append(
            {"xt": xt, "wqk": wqk.astype(bf16), "wv": wv.astype(bf16),
             "wot": wot, "maskp": maskp, "smalls": smalls}
        )

    res = _run_spmd(nc, in_maps)
    out = np.zeros((S, D), dtype=np.float32)
    for h in range(H):
        out += np.asarray(res.results[h]["out_partial"], dtype=np.float32)
    out += bo
    return out.reshape(1, S, D)


def _run_spmd(nc, in_maps, **kw):
    from concourse.bass_utils import run_bass_kernel_spmd

    return run_bass_kernel_spmd(nc, in_maps, list(range(H)), **kw)


# revision 21
# speedup vs baseline: 1.4990x; 1.0781x over previous
"""Bar-level attention Trainium2 kernel (8 NeuronCores, head-parallel).

Contract: kernel(**inputs) takes the FULL inputs from setup_inputs() and
returns the FULL [1, 2048, 512] float32 output.

Strategy (one head per core, 8 heads / 8 cores), all matmul IO in bf16
(PSUM accumulation stays fp32):
  - Host: XT [512, 2048] bf16; per-head packs:
      wqk [128, 4*128]: per 128-row contraction chunk kc, cols 0:64 =
        (Wq_h.T * scale)[kc], cols 64:128 = Wk_h.T[kc]  -> Q^T and K^T come
        out of ONE matmul stream (stacked stationary, 128 out rows).
      wv  [128, 4*64]: Wv_h.T chunks (V computed in [key, dh] layout with
        64-wide moving operand).
      wot2 [128, 512]: rows 0:64 = g*Wo_h.T, rows 64:128 = (1-g)*Wo_h.T
        (gate folded into the output projection).
      maskp: per-key-chunk bar-equality bands, packed to their true widths.
  - Device per core:
      warmup dummy matmuls (PE p-state ramp), projections pipelined under
      the XT DMA (per-contraction-chunk accumulation passes), then per
      query half: scores S^T = K_c^T Q (keys on partitions), Exp on Act
      engine (the critical resource: ~33us of column time), global AV and
      masked local AV accumulate in PSUM with a trailing ones column giving
      softmax denominators for free.  PSUM has_written semantics (start=True
      clears the whole bank; cleared words are overwritten, not
      accumulated) let local AV pieces accumulate without zero-init.
      Transition: Pool broadcasts the denominator rows, DVE divides the AV
      rows and stacks local (rows 0:64) over global (rows 64:128) in bf16;
      output projection is then ONE matmul per 128-query chunk against
      wot2, drained round-robin over Act/DVE/Pool into bf16 and DMA'd out.
  - Host: sum the 8 bf16 partials in fp32 (contraction-sharded Wo) + bo.

The global-attention additive bias in the reference is per-query and
softmax is shift-invariant per row, so it drops out exactly.
"""

import numpy as np

S = 2048
D = 512
H = 8
DH = 64
SCALE = 1.0 / np.sqrt(DH)
NCHUNK = S // 128       # 16 key chunks of 128
NHALF = 2               # query halves of 1024
QHALF = S // NHALF
VSTRIDE = 66            # per-chunk stride in the packed V tile (64 + ones + pad)


def _legalize_waits(nc, mybir):
    """This walrus codegen accepts at most ONE sync wait per instruction.
    Split any instruction carrying N>1 waits into N-1 preceding single-wait
    NoOps on the same engine (waits execute in order on the sequencer)."""
    ctr = 0
    for f in nc.m.functions:
        for b in f.blocks:
            insts = b.instructions
            if not any(i.sync_info and len(i.sync_info.on_wait) > 1 for i in insts):
                continue
            new = []
            for ins in insts:
                si = ins.sync_info
                if si is not None and len(si.on_wait) > 1:
                    waits = list(si.on_wait)
                    for w in waits[:-1]:
                        ctr += 1
                        nop = mybir.InstNoOp(name=f"waitsplit-{ctr}", engine=ins.engine)
                        nop.sync_info = mybir.SyncInfo(on_wait=[w], on_update=[])
                        new.append(nop)
                    ins.sync_info = mybir.SyncInfo(
                        on_wait=[waits[-1]], on_update=list(si.on_update)
                    )
                new.append(ins)
            insts.clear()
            insts.extend(new)
    return ctr


def _bar_bounds(bp):
    """bp: sorted int array [S] -> list of (start, end) per bar."""
    change = np.nonzero(np.diff(bp))[0] + 1
    starts = np.concatenate([[0], change])
    ends = np.concatenate([change, [len(bp)]])
    return list(zip(starts.tolist(), ends.tolist()))


def _attn_layout(bars):
    """Static layout derived from the (baked) bar boundaries.

    band[c]  = (blo, bhi): union query span of bars intersecting key chunk c
    moff[c]  = column offset of chunk c's band in the packed mask tile
    segs[(hq,c)] = (hs, he) band clipped to the query half, or None
    splits[(hq,c)] = [(a, b, start, stop)]: seg split at 512-col PSUM bank
      boundaries; start/stop mark the first/last matmul touching each bank
      of the local-AV accumulator (has_written bank epoch management).
    """
    band = []
    for c in range(NCHUNK):
        klo, khi = c * 128, (c + 1) * 128
        bs = [b for b in bars if b[1] > klo and b[0] < khi]
        blo, bhi = bs[0][0], bs[-1][1]
        if bhi - blo > 512:
            return None
        band.append((blo, bhi))
    widths = [(b[1] - b[0] + 1) // 2 * 2 for b in band]  # pad even
    moff = [0] * NCHUNK
    for c in range(1, NCHUNK):
        moff[c] = moff[c - 1] + widths[c - 1]
    mw = moff[-1] + widths[-1]

    segs = {}
    splits = {}
    for hq in range(NHALF):
        qlo, qhi = hq * QHALF, (hq + 1) * QHALF
        bank_touch = {}
        for c in range(NCHUNK):
            blo, bhi = band[c]
            hs, he = max(blo, qlo), min(bhi, qhi)
            if hs >= he:
                segs[(hq, c)] = None
                continue
            segs[(hq, c)] = (hs, he)
            ss = []
            a = hs
            while a < he:
                b = min(he, qlo + ((a - qlo) // 512 + 1) * 512)
                bank_touch.setdefault((a - qlo) // 512, []).append((c, len(ss)))
                ss.append([a, b, False, False])
                a = b
            splits[(hq, c)] = ss
        for _, lst in bank_touch.items():
            c0, i0 = lst[0]
            splits[(hq, c0)][i0][2] = True
            c1, i1 = lst[-1]
            splits[(hq, c1)][i1][3] = True
    return band, moff, mw, segs, splits


def _build(bars):
    import concourse.bass as bass
    import concourse.tile as tile
    import concourse.mybir as mybir

    dt = mybir.dt
    AF = mybir.ActivationFunctionType
    OP = mybir.AluOpType
    f32 = dt.float32
    f32r = dt.float32r
    bf16 = dt.bfloat16

    lay = _attn_layout(bars)
    assert lay is not None
    band, moff, mw, segs, splits = lay

    nc = bass.Bass()
    xt_d = nc.dram_tensor("xt", [D, S], bf16, kind="ExternalInput")
    wqk_d = nc.dram_tensor("wqk", [128, 4 * 128], bf16, kind="ExternalInput")
    wv_d = nc.dram_tensor("wv", [128, 4 * 64], bf16, kind="ExternalInput")
    wot_d = nc.dram_tensor("wot", [DH, D], f32r, kind="ExternalInput")
    maskp_d = nc.dram_tensor("maskp", [128, mw], bf16, kind="ExternalInput")
    smalls_d = nc.dram_tensor("smalls", [128, 4], f32, kind="ExternalInput")
    out_d = nc.dram_tensor("out_partial", [S, D], bf16, kind="ExternalOutput")

    with tile.TileContext(nc, pool_alloc_mode="queue") as tc:
        with tc.tile_pool(name="persist", bufs=1) as p_keep:
            qt = p_keep.tile([DH, S], bf16, tag="qt")
            kt = p_keep.tile([DH, S], bf16, tag="kt")
            vt = p_keep.tile([128, NCHUNK * VSTRIDE], bf16, tag="vt")
            wot = p_keep.tile([DH, D], f32r, tag="wot")
            maskp = p_keep.tile([128, mw], bf16, tag="maskp")
            # smalls [128,4] f32: rows 0:64 col0 = bq*scale, col1 = bk;
            # all rows: col2 = sigmoid(gate), col3 = 1-sigmoid(gate)
            smalls = p_keep.tile([128, 4], f32, tag="smalls")
            obuf = [
                p_keep.tile([128, 4 * D], bf16, tag=f"ob{i}", name=f"obuf{i}")
                for i in range(4)
            ]
            wtiny = p_keep.tile([128, 128], bf16, tag="wtiny")
            # transposed denominators / reciprocals: cols hq*16+jj = local,
            # hq*16+8+jj = global; r2[p, hq*16+jj] = gate/l_local(q) for
            # q = hq*1024 + p*8 + jj (the stride-8 interleave makes each
            # output chunk's scales one column)
            l2 = p_keep.tile([128, 32], f32, tag="l2")
            r2 = p_keep.tile([128, 32], f32, tag="r2")
            ol_sb = [
                p_keep.tile([DH + 1, QHALF], f32r, tag=f"olsb{h}", name=f"ol_sb{h}")
                for h in range(NHALF)
            ]
            og_sb = [
                p_keep.tile([DH + 1, QHALF], f32r, tag=f"ogsb{h}", name=f"og_sb{h}")
                for h in range(NHALF)
            ]

            # ---- PE p-state warmup: keep PE busy from t~0 so the 3us ramp
            # to max clock completes under the input DMA.
            nc.gpsimd.memset(wtiny[:], 0.0)
            # ones columns of the packed V tile (col 64 of each 66-wide chunk)
            nc.gpsimd.memset(
                vt.rearrange("p (c j) -> p c j", j=VSTRIDE)[:, :, DH : DH + 1], 1.0
            )
            # warmup holds 4 banks so the score pool inherits banks with
            # no dependency on the projection drains
            with tc.tile_pool(name="pwarm", bufs=1, space="PSUM") as p_w:
                wpa = p_w.tile([128, QHALF], f32, tag="wpa")
                wpb = p_w.tile([128, QHALF], f32, tag="wpb")
                for i in range(30):
                    wp = wpa if i % 2 == 0 else wpb
                    nc.tensor.matmul(
                        wp[:, 0:128], wtiny[:], wtiny[:],
                        start=True, stop=True, skip_group_check=True,
                    )

            # ---------------- projections + attention ----------------
            # Software-pipelined: half-0 projections run under the xt DMA;
            # half-1 projection passes + drains are injected into half-0's
            # score/exp stream (Act only ever runs exp once the loop
            # starts); og/lAV accumulation for early chunks is emitted late
            # so its PSUM banks (freed by the half-1 projection pool) are
            # ready without stalling the in-order engines.
            with tc.tile_pool(name="inp", bufs=1) as p_in:
                wqk = p_in.tile([128, 4 * 128], bf16, tag="wqk")
                wv = p_in.tile([128, 4 * 64], bf16, tag="wv")
                xts = [
                    p_in.tile([128, S], bf16, tag=f"xt{i}", name=f"xts{i}")
                    for i in range(4)
                ]
                # DMA issue order == service order: half-0 panels
                # interleaved with the small weight tensors, then half-1
                # panels, then mask/wot (not consumed until later).
                nc.sync.dma_start(xts[0][:, 0:QHALF], xt_d[0:128, 0:QHALF])
                nc.sync.dma_start(wqk[:], wqk_d[:])
                nc.sync.dma_start(xts[1][:, 0:QHALF], xt_d[128:256, 0:QHALF])
                nc.sync.dma_start(wv[:], wv_d[:])
                nc.sync.dma_start(xts[2][:, 0:QHALF], xt_d[256:384, 0:QHALF])
                nc.sync.dma_start(smalls[:], smalls_d[:])
                nc.sync.dma_start(xts[3][:, 0:QHALF], xt_d[384:512, 0:QHALF])
                for kc in range(4):
                    nc.sync.dma_start(
                        xts[kc][:, QHALF:S],
                        xt_d[kc * 128 : (kc + 1) * 128, QHALF:S],
                    )
                nc.sync.dma_start(maskp[:], maskp_d[:])
                nc.sync.dma_start(wot[:], wot_d[:])

                def proj_pass(qkp, vp, h, kc):
                    hq0 = h * QHALF
                    for n in range(QHALF // 512):
                        nc.tensor.matmul(
                            qkp[:, n * 512 : (n + 1) * 512],
                            wqk[:, kc * 128 : (kc + 1) * 128],
                            xts[kc][:, hq0 + n * 512 : hq0 + (n + 1) * 512],
                            start=(kc == 0),
                            stop=(kc == 3),
                        )
                    for cc in range(8):
                        nc.tensor.matmul(
                            vp[:, cc * DH : (cc + 1) * DH],
                            xts[kc][:, hq0 + cc * 128 : hq0 + (cc + 1) * 128],
                            wv[:, kc * DH : (kc + 1) * DH],
                            start=(kc == 0 and cc == 0),
                            stop=(kc == 3),
                            skip_group_check=True,
                        )

                def proj_drain(qkp, vp, h, qt_eng_act):
                    hq0 = h * QHALF
                    if qt_eng_act:
                        nc.scalar.activation(
                            qt[:, hq0 : hq0 + QHALF], qkp[0:DH, :],
                            AF.Identity, bias=smalls[0:DH, 0:1],
                        )
                    else:
                        nc.vector.tensor_scalar_add(
                            qt[:, hq0 : hq0 + QHALF], qkp[0:DH, :],
                            smalls[0:DH, 0:1],
                        )
                    nc.vector.tensor_scalar_add(
                        kt[:, hq0 : hq0 + QHALF], qkp[DH:128, :],
                        smalls[0:DH, 1:2],
                    )
                    dstv = vt[
                        :, h * 8 * VSTRIDE : (h + 1) * 8 * VSTRIDE
                    ].rearrange("p (c j) -> p c j", j=VSTRIDE)[:, :, 0:DH]
                    nc.vector.tensor_copy(
                        dstv, vp[:].rearrange("p (c j) -> p c j", j=DH)
                    )

                with tc.tile_pool(name="ph0", bufs=1, space="PSUM") as ph0:
                    qk0 = ph0.tile([128, QHALF], f32, tag="qk0")
                    v0 = ph0.tile([128, 8 * DH], f32, tag="v0")
                    for kc in range(4):
                        proj_pass(qk0, v0, 0, kc)
                    proj_drain(qk0, v0, 0, qt_eng_act=True)

                with (
                    tc.tile_pool(name="ps", bufs=2, space="PSUM") as p_s,
                    tc.tile_pool(name="pe", bufs=12) as p_e,
                    tc.tile_pool(name="pel", bufs=12) as p_el,
                ):
                    es = {}
                    els = {}

                    def emit_sc_exp(hq, c):
                        qlo = hq * QHALF
                        sc = p_s.tile([128, QHALF], f32, tag="s", name="sc")
                        for n in range(QHALF // 512):
                            nc.tensor.matmul(
                                sc[:, n * 512 : (n + 1) * 512],
                                kt[:, c * 128 : (c + 1) * 128],
                                qt[:, qlo + n * 512 : qlo + (n + 1) * 512],
                                start=True,
                                stop=True,
                            )
                        ec = p_e.tile([128, QHALF], bf16, tag="e", name="ec")
                        nc.scalar.activation(ec[:], sc[:], AF.Exp)
                        es[(hq, c)] = ec
                        seg = segs[(hq, c)]
                        if seg is not None:
                            hs, he = seg
                            blo = band[c][0]
                            el = p_el.tile([128, 512], bf16, tag="el", name="el")
                            nc.vector.tensor_mul(
                                el[:, 0 : he - hs],
                                ec[:, hs - qlo : he - qlo],
                                maskp[:, moff[c] + hs - blo : moff[c] + he - blo],
                            )
                            els[(hq, c)] = el

                    def emit_og_lav(hq, c, og, ol):
                        qlo = hq * QHALF
                        ec = es.pop((hq, c))
                        vst = vt[:, c * VSTRIDE : c * VSTRIDE + DH + 1]
                        for n in range(QHALF // 512):
                            nc.tensor.matmul(
                                og[:, n * 512 : (n + 1) * 512],
                                vst,
                                ec[:, n * 512 : (n + 1) * 512],
                                start=(c == 0),
                                stop=(c == NCHUNK - 1),
                            )
                        if (hq, c) in els:
                            el = els.pop((hq, c))
                            hs = segs[(hq, c)][0]
                            for (a, b, st, sp) in splits[(hq, c)]:
                                nc.tensor.matmul(
                                    ol[:, a - qlo : b - qlo],
                                    vst,
                                    el[:, a - hs : b - hs],
                                    start=st,
                                    stop=sp,
                                    skip_group_check=True,
                                )

                    def transition(hq, og, ol):
                        nc.vector.tensor_copy(ol_sb[hq][:], ol[:])
                        nc.vector.tensor_copy(og_sb[hq][:], og[:])
                        c0 = hq * 16
                        nc.sync.dma_start(
                            l2[:, c0 : c0 + 8],
                            ol_sb[hq][DH : DH + 1, :].bitcast(f32),
                        )
                        nc.sync.dma_start(
                            l2[:, c0 + 8 : c0 + 16],
                            og_sb[hq][DH : DH + 1, :].bitcast(f32),
                        )
                        nc.vector.reciprocal(
                            r2[:, c0 : c0 + 16], l2[:, c0 : c0 + 16]
                        )
                        nc.vector.tensor_scalar_mul(
                            r2[:, c0 : c0 + 8], r2[:, c0 : c0 + 8],
                            smalls[:, 2:3],
                        )
                        nc.vector.tensor_scalar_mul(
                            r2[:, c0 + 8 : c0 + 16], r2[:, c0 + 8 : c0 + 16],
                            smalls[:, 3:4],
                        )

                    with tc.tile_pool(name="ph1", bufs=1, space="PSUM") as ph1:
                        qk1 = ph1.tile([128, QHALF], f32, tag="qk1")
                        v1 = ph1.tile([128, 8 * DH], f32, tag="v1")
                        # chunks 0..7 only: their kt key-columns come from
                        # the half-0 projection; chunks 8..15 need the half-1
                        # drain below first
                        for c in range(8):
                            emit_sc_exp(0, c)
                            if 1 <= c <= 4:
                                proj_pass(qk1, v1, 1, c - 1)
                        proj_drain(qk1, v1, 1, qt_eng_act=False)

                    with (
                        tc.tile_pool(name="pog", bufs=1, space="PSUM") as p_og,
                        tc.tile_pool(name="pol", bufs=1, space="PSUM") as p_ol,
                    ):
                        og0 = p_og.tile([DH + 1, QHALF], f32, tag="og", name="og0")
                        ol0 = p_ol.tile([DH + 1, QHALF], f32, tag="ol", name="ol0")
                        for c in range(8):
                            emit_sc_exp(0, 8 + c)
                            emit_og_lav(0, c, og0, ol0)
                        for c in range(8, NCHUNK):
                            emit_og_lav(0, c, og0, ol0)
                        transition(0, og0, ol0)

                        og1 = p_og.tile([DH + 1, QHALF], f32, tag="og", name="og1")
                        ol1 = p_ol.tile([DH + 1, QHALF], f32, tag="ol", name="ol1")
                        for c in range(NCHUNK):
                            emit_sc_exp(1, c)
                            if c >= 2:
                                emit_og_lav(1, c - 2, og1, ol1)
                        for c in range(NCHUNK - 2, NCHUNK):
                            emit_og_lav(1, c, og1, ol1)
                        transition(1, og1, ol1)

            # ---------------- output projection ----------------
            # interleaved query chunks (queries jj, jj+8, ...): partition p
            # of chunk jj is query hq*1024 + p*8 + jj, so r2 column
            # hq*16+jj is exactly the per-partition scale vector.  Per
            # chunk: local+global projections into one 2-bank psum pair,
            # t1 = lp*r_l on Act (activation scale), out = gp*r_g + t1 on
            # DVE (scalar_tensor_tensor), both reading PSUM directly.
            with (
                tc.tile_pool(name="pout", bufs=4, space="PSUM") as p_o,
                tc.tile_pool(name="pt1", bufs=4) as p_t1,
            ):
                for hq in range(NHALF):
                    c0 = hq * 16
                    for jj in range(8):
                        op = p_o.tile([128, 2 * D], f32, tag="op")
                        nc.tensor.matmul(
                            op[:, 0:D],
                            ol_sb[hq][0:DH, jj:QHALF:8],
                            wot[:],
                            start=True,
                            stop=True,
                        )
                        nc.tensor.matmul(
                            op[:, D : 2 * D],
                            og_sb[hq][0:DH, jj:QHALF:8],
                            wot[:],
                            start=True,
                            stop=True,
                        )
                        t1 = p_t1.tile([128, D], bf16, tag="t1")
                        nc.scalar.activation(
                            t1[:], op[:, 0:D], AF.Identity,
                            scale=r2[:, c0 + jj : c0 + jj + 1],
                        )
                        grp = obuf[(hq * 8 + jj) // 4]
                        nc.vector.scalar_tensor_tensor(
                            grp[:, (jj % 4) * D : (jj % 4 + 1) * D],
                            op[:, D : 2 * D],
                            r2[:, c0 + 8 + jj : c0 + 9 + jj],
                            t1[:],
                            OP.mult,
                            OP.add,
                        )
                        if jj % 4 == 3:
                            dst = out_d[
                                hq * QHALF : (hq + 1) * QHALF, :
                            ].rearrange("(p j) c -> p j c", j=8)[:, jj - 3 : jj + 1, :]
                            nc.sync.dma_start(
                                dst, grp[:].rearrange("p (j c) -> p j c", j=4)
                            )

    _legalize_waits(nc, mybir)
    return nc


_CACHE = {}


def _get_built(bar_key, bars):
    if bar_key not in _CACHE:
        _CACHE[bar_key] = _build(bars)
    return _CACHE[bar_key]


def _np_reference(hidden_states, bar_positions, attention_mask, Wq, bq, Wk, bk,
                  Wv, bv, Wo, bo, bar_emb, gate):
    """Plain numpy fallback (only used if inputs violate baked assumptions)."""
    B, S_, _ = hidden_states.shape
    x = hidden_states.astype(np.float64)
    q = (x @ Wq.T + bq).reshape(B, S_, H, DH).transpose(0, 2, 1, 3)
    k = (x @ Wk.T + bk).reshape(B, S_, H, DH).transpose(0, 2, 1, 3)
    v = (x @ Wv.T + bv).reshape(B, S_, H, DH).transpose(0, 2, 1, 3)
    scores = np.einsum("bhqd,bhkd->bhqk", q, k) * SCALE
    pad = attention_mask[:, None, None, :]
    bar_mask = (bar_positions[:, :, None] == bar_positions[:, None, :])[:, None]
    NEG = -np.inf

    def softmax(s):
        s = s - s.max(-1, keepdims=True)
        e = np.exp(s)
        return e / e.sum(-1, keepdims=True)

    local = softmax(np.where(bar_mask & pad, scores, NEG))
    emb = bar_emb[np.asarray(bar_positions) % bar_emb.shape[0]]
    bias = np.sum(emb * emb, axis=-1)
    glob = softmax(np.where(pad, scores + bias[:, None, :, None], NEG))
    la = np.einsum("bhqk,bhkd->bhqd", local, v)
    ga = np.einsum("bhqk,bhkd->bhqd", glob, v)
    g = 1.0 / (1.0 + np.exp(-gate))[None, :, None, None]
    comb = g * la + (1.0 - g) * ga
    out = comb.transpose(0, 2, 1, 3).reshape(B, S_, H * DH)
    return (out @ Wo.T + bo).astype(np.float32)


def kernel(**inputs):
    import ml_dtypes

    bf16 = ml_dtypes.bfloat16

    hidden_states = np.asarray(inputs["hidden_states"], dtype=np.float32)
    bar_positions = np.asarray(inputs["bar_positions"])
    attention_mask = np.asarray(inputs["attention_mask"])
    Wq = np.asarray(inputs["Wq"], dtype=np.float32)
    bq = np.asarray(inputs["bq"], dtype=np.float32)
    Wk = np.asarray(inputs["Wk"], dtype=np.float32)
    bk = np.asarray(inputs["bk"], dtype=np.float32)
    Wv = np.asarray(inputs["Wv"], dtype=np.float32)
    bv = np.asarray(inputs["bv"], dtype=np.float32)
    Wo = np.asarray(inputs["Wo"], dtype=np.float32)
    bo = np.asarray(inputs["bo"], dtype=np.float32)
    gate = np.asarray(inputs["gate"], dtype=np.float32)

    bp = bar_positions[0].astype(np.int64)
    bars = _bar_bounds(bp)
    if (
        hidden_states.shape != (1, S, D)
        or not bool(attention_mask.all())
        or not bool((np.diff(bp) >= 0).all())
        or bool(np.abs(bv).max() > 0)
        or _attn_layout(bars) is None
    ):
        return _np_reference(
            hidden_states, bar_positions, attention_mask, Wq, bq, Wk, bk,
            Wv, bv, Wo, bo, np.asarray(inputs["bar_emb"], dtype=np.float32), gate,
        )

    nc = _get_built(bp.tobytes(), bars)
    band, moff, mw, _, _ = _attn_layout(bars)

    # packed mask bands (same for every core)
    maskp = np.zeros((128, mw), dtype=bf16)
    for c in range(NCHUNK):
        klo, khi = c * 128, (c + 1) * 128
        blo, bhi = band[c]
        eq = bp[klo:khi, None] == bp[None, blo:bhi]
        maskp[:, moff[c] : moff[c] + (bhi - blo)] = eq.astype(bf16)

    xt = np.ascontiguousarray(hidden_states[0].T).astype(bf16)  # [512, 2048]
    g = 1.0 / (1.0 + np.exp(-gate.astype(np.float64)))  # sigmoid, [H]
    in_maps = []
    for h in range(H):
        sl = slice(h * DH, (h + 1) * DH)
        wqt = Wq[sl, :].T * np.float32(SCALE)  # [512, 64]
        wkt = Wk[sl, :].T
        wvt = Wv[sl, :].T
        wqk = np.empty((128, 4 * 128), dtype=np.float32)
        wv = np.empty((128, 4 * 64), dtype=np.float32)
        for kc in range(4):
            r = slice(kc * 128, (kc + 1) * 128)
            wqk[:, kc * 128 : kc * 128 + 64] = wqt[r]
            wqk[:, kc * 128 + 64 : (kc + 1) * 128] = wkt[r]
            wv[:, kc * 64 : (kc + 1) * 64] = wvt[r]
        wot = np.ascontiguousarray(Wo[:, sl].T)  # [64, 512] fp32
        smalls = np.zeros((128, 4), dtype=np.float32)
        smalls[0:DH, 0] = bq[sl] * np.float32(SCALE)
        smalls[0:DH, 1] = bk[sl]
        smalls[:, 2] = np.float32(g[h])
        smalls[:, 3] = np.float32(1.0 - g[h])
        in_maps.append(
            {"xt": xt, "wqk": wqk.astype(bf16), "wv": wv.astype(bf16),
             "wot": wot, "maskp": maskp, "smalls": smalls}
        )

    res = _run_spmd(nc, in_maps)
    out = np.zeros((S, D), dtype=np.float32)
    for h in range(H):
        out += np.asarray(res.results[h]["out_partial"], dtype=np.float32)
    out += bo
    return out.reshape(1, S, D)


def _run_spmd(nc, in_maps, **kw):
    from concourse.bass_utils import run_bass_kernel_spmd

    return run_bass_kernel_spmd(nc, in_maps, list(range(H)), **kw)


# revision 23
# speedup vs baseline: 1.5301x; 1.0208x over previous
"""Bar-level attention Trainium2 kernel (8 NeuronCores, head-parallel).

Contract: kernel(**inputs) takes the FULL inputs from setup_inputs() and
returns the FULL [1, 2048, 512] float32 output.

Strategy (one head per core, 8 heads / 8 cores), all matmul IO in bf16
(PSUM accumulation stays fp32):
  - Host: XT [512, 2048] bf16; per-head packs:
      wqk [128, 4*128]: per 128-row contraction chunk kc, cols 0:64 =
        (Wq_h.T * scale)[kc], cols 64:128 = Wk_h.T[kc]  -> Q^T and K^T come
        out of ONE matmul stream (stacked stationary, 128 out rows).
      wv  [128, 4*64]: Wv_h.T chunks (V computed in [key, dh] layout with
        64-wide moving operand).
      wot2 [128, 512]: rows 0:64 = g*Wo_h.T, rows 64:128 = (1-g)*Wo_h.T
        (gate folded into the output projection).
      maskp: per-key-chunk bar-equality bands, packed to their true widths.
  - Device per core:
      warmup dummy matmuls (PE p-state ramp), projections pipelined under
      the XT DMA (per-contraction-chunk accumulation passes), then per
      query half: scores S^T = K_c^T Q (keys on partitions), Exp on Act
      engine (the critical resource: ~33us of column time), global AV and
      masked local AV accumulate in PSUM with a trailing ones column giving
      softmax denominators for free.  PSUM has_written semantics (start=True
      clears the whole bank; cleared words are overwritten, not
      accumulated) let local AV pieces accumulate without zero-init.
      Transition: Pool broadcasts the denominator rows, DVE divides the AV
      rows and stacks local (rows 0:64) over global (rows 64:128) in bf16;
      output projection is then ONE matmul per 128-query chunk against
      wot2, drained round-robin over Act/DVE/Pool into bf16 and DMA'd out.
  - Host: sum the 8 bf16 partials in fp32 (contraction-sharded Wo) + bo.

The global-attention additive bias in the reference is per-query and
softmax is shift-invariant per row, so it drops out exactly.
"""

import numpy as np

S = 2048
D = 512
H = 8
DH = 64
SCALE = 1.0 / np.sqrt(DH)
NCHUNK = S // 128       # 16 key chunks of 128
NHALF = 2               # query halves of 1024
QHALF = S // NHALF
VSTRIDE = 66            # per-chunk stride in the packed V tile (64 + ones + pad)


def _legalize_waits(nc, mybir):
    """This walrus codegen accepts at most ONE sync wait per instruction.
    Split any instruction carrying N>1 waits into N-1 preceding single-wait
    NoOps on the same engine (waits execute in order on the sequencer)."""
    ctr = 0
    for f in nc.m.functions:
        for b in f.blocks:
            insts = b.instructions
            if not any(i.sync_info and len(i.sync_info.on_wait) > 1 for i in insts):
                continue
            new = []
            for ins in insts:
                si = ins.sync_info
                if si is not None and len(si.on_wait) > 1:
                    waits = list(si.on_wait)
                    for w in waits[:-1]:
                        ctr += 1
                        nop = mybir.InstNoOp(name=f"waitsplit-{ctr}", engine=ins.engine)
                        nop.sync_info = mybir.SyncInfo(on_wait=[w], on_update=[])
                        new.append(nop)
                    ins.sync_info = mybir.SyncInfo(
                        on_wait=[waits[-1]], on_update=list(si.on_update)
                    )
                new.append(ins)
            insts.clear()
            insts.extend(new)
    return ctr


def _bar_bounds(bp):
    """bp: sorted int array [S] -> list of (start, end) per bar."""
    change = np.nonzero(np.diff(bp))[0] + 1
    starts = np.concatenate([[0], change])
    ends = np.concatenate([change, [len(bp)]])
    return list(zip(starts.tolist(), ends.tolist()))


def _attn_layout(bars):
    """Static layout derived from the (baked) bar boundaries.

    band[c]  = (blo, bhi): union query span of bars intersecting key chunk c
    moff[c]  = column offset of chunk c's band in the packed mask tile
    segs[(hq,c)] = (hs, he) band clipped to the query half, or None
    splits[(hq,c)] = [(a, b, start, stop)]: seg split at 512-col PSUM bank
      boundaries; start/stop mark the first/last matmul touching each bank
      of the local-AV accumulator (has_written bank epoch management).
    """
    band = []
    for c in range(NCHUNK):
        klo, khi = c * 128, (c + 1) * 128
        bs = [b for b in bars if b[1] > klo and b[0] < khi]
        blo, bhi = bs[0][0], bs[-1][1]
        if bhi - blo > 512:
            return None
        band.append((blo, bhi))
    widths = [(b[1] - b[0] + 1) // 2 * 2 for b in band]  # pad even
    moff = [0] * NCHUNK
    for c in range(1, NCHUNK):
        moff[c] = moff[c - 1] + widths[c - 1]
    mw = moff[-1] + widths[-1]

    segs = {}
    splits = {}
    for hq in range(NHALF):
        qlo, qhi = hq * QHALF, (hq + 1) * QHALF
        bank_touch = {}
        for c in range(NCHUNK):
            blo, bhi = band[c]
            hs, he = max(blo, qlo), min(bhi, qhi)
            if hs >= he:
                segs[(hq, c)] = None
                continue
            segs[(hq, c)] = (hs, he)
            ss = []
            a = hs
            while a < he:
                b = min(he, qlo + ((a - qlo) // 512 + 1) * 512)
                bank_touch.setdefault((a - qlo) // 512, []).append((c, len(ss)))
                ss.append([a, b, False, False])
                a = b
            splits[(hq, c)] = ss
        for _, lst in bank_touch.items():
            c0, i0 = lst[0]
            splits[(hq, c0)][i0][2] = True
            c1, i1 = lst[-1]
            splits[(hq, c1)][i1][3] = True
    return band, moff, mw, segs, splits


def _build(bars):
    import concourse.bass as bass
    import concourse.tile as tile
    import concourse.mybir as mybir

    dt = mybir.dt
    AF = mybir.ActivationFunctionType
    OP = mybir.AluOpType
    f32 = dt.float32
    f32r = dt.float32r
    bf16 = dt.bfloat16

    lay = _attn_layout(bars)
    assert lay is not None
    band, moff, mw, segs, splits = lay

    nc = bass.Bass()
    xt_d = nc.dram_tensor("xt", [D, S], bf16, kind="ExternalInput")
    wqk_d = nc.dram_tensor("wqk", [128, 4 * 128], bf16, kind="ExternalInput")
    wv_d = nc.dram_tensor("wv", [128, 4 * 64], bf16, kind="ExternalInput")
    wot_d = nc.dram_tensor("wot", [DH, D], f32r, kind="ExternalInput")
    maskp_d = nc.dram_tensor("maskp", [128, mw], bf16, kind="ExternalInput")
    smalls_d = nc.dram_tensor("smalls", [128, 4], f32, kind="ExternalInput")
    out_d = nc.dram_tensor("out_partial", [S, D], bf16, kind="ExternalOutput")

    with tile.TileContext(nc, pool_alloc_mode="queue") as tc:
        with tc.tile_pool(name="persist", bufs=1) as p_keep:
            qt = p_keep.tile([DH, S], bf16, tag="qt")
            kt = p_keep.tile([DH, S], bf16, tag="kt")
            vt = p_keep.tile([128, NCHUNK * VSTRIDE], bf16, tag="vt")
            wot = p_keep.tile([DH, D], f32r, tag="wot")
            maskp = p_keep.tile([128, mw], bf16, tag="maskp")
            # smalls [128,4] f32: rows 0:64 col0 = bq*scale, col1 = bk;
            # all rows: col2 = sigmoid(gate), col3 = 1-sigmoid(gate)
            smalls = p_keep.tile([128, 4], f32, tag="smalls")
            obuf = [
                p_keep.tile([128, 2 * D], bf16, tag=f"ob{i}", name=f"obuf{i}")
                for i in range(8)
            ]
            wtiny = p_keep.tile([128, 128], bf16, tag="wtiny")
            # transposed denominators / reciprocals: cols hq*16+jj = local,
            # hq*16+8+jj = global; r2[p, hq*16+jj] = gate/l_local(q) for
            # q = hq*1024 + p*8 + jj (the stride-8 interleave makes each
            # output chunk's scales one column)
            l2 = p_keep.tile([128, 32], f32, tag="l2")
            r2 = p_keep.tile([128, 32], f32, tag="r2")
            ol_sb = [
                p_keep.tile([DH + 1, QHALF], f32r, tag=f"olsb{h}", name=f"ol_sb{h}")
                for h in range(NHALF)
            ]
            og_sb = [
                p_keep.tile([DH + 1, QHALF], f32r, tag=f"ogsb{h}", name=f"og_sb{h}")
                for h in range(NHALF)
            ]

            # ---- PE p-state warmup: keep PE busy from t~0 so the 3us ramp
            # to max clock completes under the input DMA.
            nc.gpsimd.memset(wtiny[:], 0.0)
            # ones columns of the packed V tile (col 64 of each 66-wide chunk)
            nc.gpsimd.memset(
                vt.rearrange("p (c j) -> p c j", j=VSTRIDE)[:, :, DH : DH + 1], 1.0
            )
            # warmup holds 4 banks so the score pool inherits banks with
            # no dependency on the projection drains
            with tc.tile_pool(name="pwarm", bufs=1, space="PSUM") as p_w:
                wpa = p_w.tile([128, QHALF], f32, tag="wpa")
                wpb = p_w.tile([128, QHALF], f32, tag="wpb")
                for i in range(30):
                    wp = wpa if i % 2 == 0 else wpb
                    nc.tensor.matmul(
                        wp[:, 0:128], wtiny[:], wtiny[:],
                        start=True, stop=True, skip_group_check=True,
                    )

            # ---------------- projections + attention ----------------
            # Software-pipelined: half-0 projections run under the xt DMA;
            # half-1 projection passes + drains are injected into half-0's
            # score/exp stream (Act only ever runs exp once the loop
            # starts); og/lAV accumulation for early chunks is emitted late
            # so its PSUM banks (freed by the half-1 projection pool) are
            # ready without stalling the in-order engines.
            with tc.tile_pool(name="inp", bufs=1) as p_in:
                wqk = p_in.tile([128, 4 * 128], bf16, tag="wqk")
                wv = p_in.tile([128, 4 * 64], bf16, tag="wv")
                xts = [
                    p_in.tile([128, S], bf16, tag=f"xt{i}", name=f"xts{i}")
                    for i in range(4)
                ]
                # DMA issue order == service order: first-needed first.
                nc.sync.dma_start(xts[0][:, 0:QHALF], xt_d[0:128, 0:QHALF])
                nc.sync.dma_start(wqk[:], wqk_d[:])
                nc.sync.dma_start(smalls[:], smalls_d[:])
                nc.sync.dma_start(xts[1][:, 0:QHALF], xt_d[128:256, 0:QHALF])
                nc.sync.dma_start(xts[2][:, 0:QHALF], xt_d[256:384, 0:QHALF])
                nc.sync.dma_start(xts[3][:, 0:QHALF], xt_d[384:512, 0:QHALF])
                nc.sync.dma_start(wv[:], wv_d[:])
                nc.sync.dma_start(maskp[:], maskp_d[:])
                for kc in range(4):
                    nc.sync.dma_start(
                        xts[kc][:, QHALF:S],
                        xt_d[kc * 128 : (kc + 1) * 128, QHALF:S],
                    )
                nc.sync.dma_start(wot[:], wot_d[:])

                def qk_pass(qkp, h, kc):
                    hq0 = h * QHALF
                    for n in range(QHALF // 512):
                        nc.tensor.matmul(
                            qkp[:, n * 512 : (n + 1) * 512],
                            wqk[:, kc * 128 : (kc + 1) * 128],
                            xts[kc][:, hq0 + n * 512 : hq0 + (n + 1) * 512],
                            start=(kc == 0),
                            stop=(kc == 3),
                        )

                def v_pass(vp, h, kc):
                    hq0 = h * QHALF
                    for cc in range(8):
                        nc.tensor.matmul(
                            vp[:, cc * DH : (cc + 1) * DH],
                            xts[kc][:, hq0 + cc * 128 : hq0 + (cc + 1) * 128],
                            wv[:, kc * DH : (kc + 1) * DH],
                            start=(kc == 0 and cc == 0),
                            stop=(kc == 3),
                            skip_group_check=True,
                        )

                def v_drain(vp, h):
                    dstv = vt[
                        :, h * 8 * VSTRIDE : (h + 1) * 8 * VSTRIDE
                    ].rearrange("p (c j) -> p c j", j=VSTRIDE)[:, :, 0:DH]
                    nc.vector.tensor_copy(
                        dstv, vp[:].rearrange("p (c j) -> p c j", j=DH)
                    )

                with (
                    tc.tile_pool(name="ps", bufs=2, space="PSUM") as p_s,
                    tc.tile_pool(name="pe", bufs=12) as p_e,
                    tc.tile_pool(name="pel", bufs=12) as p_el,
                ):
                    es = {}
                    els = {}

                    def emit_sc_exp(hq, c):
                        qlo = hq * QHALF
                        sc = p_s.tile([128, QHALF], f32, tag="s", name="sc")
                        for n in range(QHALF // 512):
                            nc.tensor.matmul(
                                sc[:, n * 512 : (n + 1) * 512],
                                kt[:, c * 128 : (c + 1) * 128],
                                qt[:, qlo + n * 512 : qlo + (n + 1) * 512],
                                start=True,
                                stop=True,
                            )
                        ec = p_e.tile([128, QHALF], bf16, tag="e", name="ec")
                        nc.scalar.activation(ec[:], sc[:], AF.Exp)
                        es[(hq, c)] = ec
                        seg = segs[(hq, c)]
                        if seg is not None:
                            hs, he = seg
                            blo = band[c][0]
                            el = p_el.tile([128, 512], bf16, tag="el", name="el")
                            nc.vector.tensor_mul(
                                el[:, 0 : he - hs],
                                ec[:, hs - qlo : he - qlo],
                                maskp[:, moff[c] + hs - blo : moff[c] + he - blo],
                            )
                            els[(hq, c)] = el

                    def emit_og_lav(hq, c, og, ol):
                        qlo = hq * QHALF
                        ec = es.pop((hq, c))
                        vst = vt[:, c * VSTRIDE : c * VSTRIDE + DH + 1]
                        for n in range(QHALF // 512):
                            nc.tensor.matmul(
                                og[:, n * 512 : (n + 1) * 512],
                                vst,
                                ec[:, n * 512 : (n + 1) * 512],
                                start=(c == 0),
                                stop=(c == NCHUNK - 1),
                            )
                        if (hq, c) in els:
                            el = els.pop((hq, c))
                            hs = segs[(hq, c)][0]
                            for (a, b, st, sp) in splits[(hq, c)]:
                                nc.tensor.matmul(
                                    ol[:, a - qlo : b - qlo],
                                    vst,
                                    el[:, a - hs : b - hs],
                                    start=st,
                                    stop=sp,
                                    skip_group_check=True,
                                )

                    def transition(hq, og, ol):
                        # final half: og drain on Act (free there) so the
                        # PSUM banks release for the output stage sooner;
                        # mid-loop half: both on DVE to keep Act exp-only
                        nc.vector.tensor_copy(ol_sb[hq][:], ol[:])
                        if hq == NHALF - 1:
                            nc.scalar.copy(og_sb[hq][:], og[:])
                        else:
                            nc.vector.tensor_copy(og_sb[hq][:], og[:])
                        c0 = hq * 16
                        nc.sync.dma_start(
                            l2[:, c0 : c0 + 8],
                            ol_sb[hq][DH : DH + 1, :].bitcast(f32),
                        )
                        nc.sync.dma_start(
                            l2[:, c0 + 8 : c0 + 16],
                            og_sb[hq][DH : DH + 1, :].bitcast(f32),
                        )
                        nc.vector.reciprocal(
                            r2[:, c0 : c0 + 16], l2[:, c0 : c0 + 16]
                        )
                        nc.vector.tensor_scalar_mul(
                            r2[:, c0 : c0 + 8], r2[:, c0 : c0 + 8],
                            smalls[:, 2:3],
                        )
                        nc.vector.tensor_scalar_mul(
                            r2[:, c0 + 8 : c0 + 16], r2[:, c0 + 8 : c0 + 16],
                            smalls[:, 3:4],
                        )

                    with tc.tile_pool(name="ph0", bufs=1, space="PSUM") as ph0:
                        qk0 = ph0.tile([128, QHALF], f32, tag="qk0")
                        v0 = ph0.tile([128, 8 * DH], f32, tag="v0")
                        for kc in range(4):
                            qk_pass(qk0, 0, kc)
                        nc.scalar.activation(
                            qt[:, 0:QHALF], qk0[0:DH, :],
                            AF.Identity, bias=smalls[0:DH, 0:1],
                        )
                        nc.vector.tensor_scalar_add(
                            kt[:, 0:256], qk0[DH:128, 0:256], smalls[0:DH, 1:2]
                        )
                        nc.vector.tensor_scalar_add(
                            kt[:, 256:QHALF], qk0[DH:128, 256:QHALF],
                            smalls[0:DH, 1:2],
                        )
                        for kc in range(4):
                            v_pass(v0, 0, kc)
                        v_drain(v0, 0)

                    with tc.tile_pool(name="ph1", bufs=1, space="PSUM") as ph1:
                        qk1 = ph1.tile([128, QHALF], f32, tag="qk1")
                        v1 = ph1.tile([128, 8 * DH], f32, tag="v1")
                        for c in range(3):
                            emit_sc_exp(0, c)
                        for c in range(3, 7):
                            emit_sc_exp(0, c)
                            qk_pass(qk1, 1, c - 3)
                        emit_sc_exp(0, 7)
                        # kt for chunks 8..9 first so the half-0 score stream
                        # never starves, then V, then qt (only needed for the
                        # half-1 scores much later)
                        nc.vector.tensor_scalar_add(
                            kt[:, QHALF : QHALF + 256],
                            qk1[DH:128, 0:256], smalls[0:DH, 1:2],
                        )
                        nc.vector.tensor_scalar_add(
                            kt[:, QHALF + 256 : S],
                            qk1[DH:128, 256:QHALF], smalls[0:DH, 1:2],
                        )
                        for kc in range(4):
                            v_pass(v1, 1, kc)
                        v_drain(v1, 1)
                        nc.vector.tensor_scalar_add(
                            qt[:, QHALF:S], qk1[0:DH, :], smalls[0:DH, 0:1]
                        )

                    with (
                        tc.tile_pool(name="pog", bufs=1, space="PSUM") as p_og,
                        tc.tile_pool(name="pol", bufs=1, space="PSUM") as p_ol,
                    ):
                        og0 = p_og.tile([DH + 1, QHALF], f32, tag="og", name="og0")
                        ol0 = p_ol.tile([DH + 1, QHALF], f32, tag="ol", name="ol0")
                        for c in range(8):
                            emit_sc_exp(0, 8 + c)
                            emit_og_lav(0, c, og0, ol0)
                        for c in range(8, NCHUNK):
                            emit_og_lav(0, c, og0, ol0)
                        transition(0, og0, ol0)

                        og1 = p_og.tile([DH + 1, QHALF], f32, tag="og", name="og1")
                        ol1 = p_ol.tile([DH + 1, QHALF], f32, tag="ol", name="ol1")
                        for c in range(NCHUNK):
                            emit_sc_exp(1, c)
                            if c >= 2:
                                emit_og_lav(1, c - 2, og1, ol1)
                        for c in range(NCHUNK - 2, NCHUNK):
                            emit_og_lav(1, c, og1, ol1)
                        transition(1, og1, ol1)

            # ---------------- output projection ----------------
            # interleaved query chunks (queries jj, jj+8, ...): partition p
            # of chunk jj is query hq*1024 + p*8 + jj, so r2 column
            # hq*16+jj is exactly the per-partition scale vector.  Per
            # chunk: local+global projections into one 2-bank psum pair,
            # t1 = lp*r_l on Act (activation scale), out = gp*r_g + t1 on
            # DVE (scalar_tensor_tensor), both reading PSUM directly.
            with (
                tc.tile_pool(name="pout", bufs=4, space="PSUM") as p_o,
                tc.tile_pool(name="pt1", bufs=4) as p_t1,
            ):
                for hq in range(NHALF):
                    c0 = hq * 16
                    for jj in range(8):
                        op = p_o.tile([128, 2 * D], f32, tag="op")
                        nc.tensor.matmul(
                            op[:, 0:D],
                            ol_sb[hq][0:DH, jj:QHALF:8],
                            wot[:],
                            start=True,
                            stop=True,
                        )
                        nc.tensor.matmul(
                            op[:, D : 2 * D],
                            og_sb[hq][0:DH, jj:QHALF:8],
                            wot[:],
                            start=True,
                            stop=True,
                        )
                        t1 = p_t1.tile([128, D], bf16, tag="t1")
                        nc.scalar.activation(
                            t1[:], op[:, 0:D], AF.Identity,
                            scale=r2[:, c0 + jj : c0 + jj + 1],
                        )
                        grp = obuf[(hq * 8 + jj) // 2]
                        nc.vector.scalar_tensor_tensor(
                            grp[:, (jj % 2) * D : (jj % 2 + 1) * D],
                            op[:, D : 2 * D],
                            r2[:, c0 + 8 + jj : c0 + 9 + jj],
                            t1[:],
                            OP.mult,
                            OP.add,
                        )
                        if jj % 2 == 1:
                            dst = out_d[
                                hq * QHALF : (hq + 1) * QHALF, :
                            ].rearrange("(p j) c -> p j c", j=8)[:, jj - 1 : jj + 1, :]
                            nc.sync.dma_start(
                                dst, grp[:].rearrange("p (j c) -> p j c", j=2)
                            )

    _legalize_waits(nc, mybir)
    return nc


_CACHE = {}


def _get_built(bar_key, bars):
    if bar_key not in _CACHE:
        _CACHE[bar_key] = _build(bars)
    return _CACHE[bar_key]


def _np_reference(hidden_states, bar_positions, attention_mask, Wq, bq, Wk, bk,
                  Wv, bv, Wo, bo, bar_emb, gate):
    """Plain numpy fallback (only used if inputs violate baked assumptions)."""
    B, S_, _ = hidden_states.shape
    x = hidden_states.astype(np.float64)
    q = (x @ Wq.T + bq).reshape(B, S_, H, DH).transpose(0, 2, 1, 3)
    k = (x @ Wk.T + bk).reshape(B, S_, H, DH).transpose(0, 2, 1, 3)
    v = (x @ Wv.T + bv).reshape(B, S_, H, DH).transpose(0, 2, 1, 3)
    scores = np.einsum("bhqd,bhkd->bhqk", q, k) * SCALE
    pad = attention_mask[:, None, None, :]
    bar_mask = (bar_positions[:, :, None] == bar_positions[:, None, :])[:, None]
    NEG = -np.inf

    def softmax(s):
        s = s - s.max(-1, keepdims=True)
        e = np.exp(s)
        return e / e.sum(-1, keepdims=True)

    local = softmax(np.where(bar_mask & pad, scores, NEG))
    emb = bar_emb[np.asarray(bar_positions) % bar_emb.shape[0]]
    bias = np.sum(emb * emb, axis=-1)
    glob = softmax(np.where(pad, scores + bias[:, None, :, None], NEG))
    la = np.einsum("bhqk,bhkd->bhqd", local, v)
    ga = np.einsum("bhqk,bhkd->bhqd", glob, v)
    g = 1.0 / (1.0 + np.exp(-gate))[None, :, None, None]
    comb = g * la + (1.0 - g) * ga
    out = comb.transpose(0, 2, 1, 3).reshape(B, S_, H * DH)
    return (out @ Wo.T + bo).astype(np.float32)


def kernel(**inputs):
    import ml_dtypes

    bf16 = ml_dtypes.bfloat16

    hidden_states = np.asarray(inputs["hidden_states"], dtype=np.float32)
    bar_positions = np.asarray(inputs["bar_positions"])
    attention_mask = np.asarray(inputs["attention_mask"])
    Wq = np.asarray(inputs["Wq"], dtype=np.float32)
    bq = np.asarray(inputs["bq"], dtype=np.float32)
    Wk = np.asarray(inputs["Wk"], dtype=np.float32)
    bk = np.asarray(inputs["bk"], dtype=np.float32)
    Wv = np.asarray(inputs["Wv"], dtype=np.float32)
    bv = np.asarray(inputs["bv"], dtype=np.float32)
    Wo = np.asarray(inputs["Wo"], dtype=np.float32)
    bo = np.asarray(inputs["bo"], dtype=np.float32)
    gate = np.asarray(inputs["gate"], dtype=np.float32)

    bp = bar_positions[0].astype(np.int64)
    bars = _bar_bounds(bp)
    if (
        hidden_states.shape != (1, S, D)
        or not bool(attention_mask.all())
        or not bool((np.diff(bp) >= 0).all())
        or bool(np.abs(bv).max() > 0)
        or _attn_layout(bars) is None
    ):
        return _np_reference(
            hidden_states, bar_positions, attention_mask, Wq, bq, Wk, bk,
            Wv, bv, Wo, bo, np.asarray(inputs["bar_emb"], dtype=np.float32), gate,
        )

    nc = _get_built(bp.tobytes(), bars)
    band, moff, mw, _, _ = _attn_layout(bars)

    # packed mask bands (same for every core)
    maskp = np.zeros((128, mw), dtype=bf16)
    for c in range(NCHUNK):
        klo, khi = c * 128, (c + 1) * 128
        blo, bhi = band[c]
        eq = bp[klo:khi, None] == bp[None, blo:bhi]
        maskp[:, moff[c] : moff[c] + (bhi - blo)] = eq.astype(bf16)

    xt = np.ascontiguousarray(hidden_states[0].T).astype(bf16)  # [512, 2048]
    g = 1.0 / (1.0 + np.exp(-gate.astype(np.float64)))  # sigmoid, [H]
    in_maps = []
    for h in range(H):
        sl = slice(h * DH, (h + 1) * DH)
        wqt = Wq[sl, :].T * np.float32(SCALE)  # [512, 64]
        wkt = Wk[sl, :].T
        wvt = Wv[sl, :].T
        wqk = np.empty((128, 4 * 128), dtype=np.float32)
        wv = np.empty((128, 4 * 64), dtype=np.float32)
        for kc in range(4):
            r = slice(kc * 128, (kc + 1) * 128)
            wqk[:, kc * 128 : kc * 128 + 64] = wqt[r]
            wqk[:, kc * 128 + 64 : (kc + 1) * 128] = wkt[r]
            wv[:, kc * 64 : (kc + 1) * 64] = wvt[r]
        wot = np.ascontiguousarray(Wo[:, sl].T)  # [64, 512] fp32
        smalls = np.zeros((128, 4), dtype=np.float32)
        smalls[0:DH, 0] = bq[sl] * np.float32(SCALE)
        smalls[0:DH, 1] = bk[sl]
        smalls[:, 2] = np.float32(g[h])
        smalls[:, 3] = np.float32(1.0 - g[h])
        in_maps.append(
            {"xt": xt, "wqk": wqk.astype(bf16), "wv": wv.astype(bf16),
             "wot": wot, "maskp": maskp, "smalls": smalls}
        )

    res = _run_spmd(nc, in_maps)
    out = np.zeros((S, D), dtype=np.float32)
    for h in range(H):
        out += np.asarray(res.results[h]["out_partial"], dtype=np.float32)
    out += bo
    return out.reshape(1, S, D)


def _run_spmd(nc, in_maps, **kw):
    from concourse.bass_utils import run_bass_kernel_spmd

    return run_bass_kernel_spmd(nc, in_maps, list(range(H)), **kw)
